# revision 1
# baseline (speedup 1.0000x reference)
"""Trainium2 Bass kernel for a pre-LN transformer block (MHA + FFN).

Data-parallel over batch: 8 NeuronCores, one batch element each.
All matmuls run as float32r (full PE rate at free-dim>=256), storage fp32.
"""
import sys

for _p in ("/opt/trn_rl_repo", "/root/.axon_site/_ro/trn_rl_repo"):
    if _p not in sys.path:
        sys.path.insert(0, _p)

import numpy as np
import concourse.bass as bass
import concourse.tile as tile
from concourse import bacc, mybir
from concourse.bass import ds, ts
from concourse.bass_utils import run_bass_kernel_spmd
from concourse.masks import make_identity

P = 128
N = 1024          # tokens per core (seq len)
D = 1024          # d_emb
H = 16            # heads
HS = 64           # head size
FF = 4096         # ffn hidden
NT = N // P       # 8 token tiles
DB = D // P       # 8 d blocks
EBS = D // P      # 8 e blocks (qkv out features)
NH = 2            # n halves of 512
LN_EPS = 1e-5

F32 = mybir.dt.float32
R = mybir.dt.float32r
AF = mybir.ActivationFunctionType
OP = mybir.AluOpType

_CACHED_NC = None


def build_nc(use_lrelu=True):
    nc = bacc.Bacc("TRN2", target_bir_lowering=False, debug=False, num_devices=8)

    x_d = nc.dram_tensor("x", [N, D], F32, kind="ExternalInput").ap()
    wq_d = nc.dram_tensor("Wq", [H, D, HS], F32, kind="ExternalInput").ap()
    bq_d = nc.dram_tensor("bq", [H, HS], F32, kind="ExternalInput").ap()
    wk_d = nc.dram_tensor("Wk", [H, D, HS], F32, kind="ExternalInput").ap()
    bk_d = nc.dram_tensor("bk", [H, HS], F32, kind="ExternalInput").ap()
    wv_d = nc.dram_tensor("Wv", [H, D, HS], F32, kind="ExternalInput").ap()
    bv_d = nc.dram_tensor("bv", [H, HS], F32, kind="ExternalInput").ap()
    wp_d = nc.dram_tensor("Wproj", [H * HS, D], F32, kind="ExternalInput").ap()
    bp_d = nc.dram_tensor("bproj", [D], F32, kind="ExternalInput").ap()
    w1_d = nc.dram_tensor("W1", [D, FF], F32, kind="ExternalInput").ap()
    b1_d = nc.dram_tensor("b1", [FF], F32, kind="ExternalInput").ap()
    w2_d = nc.dram_tensor("W2", [FF, D], F32, kind="ExternalInput").ap()
    b2_d = nc.dram_tensor("b2", [D], F32, kind="ExternalInput").ap()
    g1_d = nc.dram_tensor("ln1_g", [D], F32, kind="ExternalInput").ap()
    c1_d = nc.dram_tensor("ln1_b", [D], F32, kind="ExternalInput").ap()
    g2_d = nc.dram_tensor("ln2_g", [D], F32, kind="ExternalInput").ap()
    c2_d = nc.dram_tensor("ln2_b", [D], F32, kind="ExternalInput").ap()
    out_d = nc.dram_tensor("out", [N, D], F32, kind="ExternalOutput").ap()
    x2pb_d = nc.dram_tensor("x2pb_scratch", [P, NT, D], F32).ap()

    with tile.TileContext(nc) as tc:
        with tc.tile_pool(name="cn", bufs=1) as cp, \
             tc.tile_pool(name="big", bufs=1) as bp:
            # ---- constants / bias vectors (persistent, tiny) ----
            ident = cp.tile([P, P], F32)
            make_identity(nc, ident[:])
            ones_f = cp.tile([P, 1], F32)
            nc.vector.memset(ones_f[:], 1.0)
            ones64 = cp.tile([1, HS], R)
            nc.vector.tensor_copy(ones64[:],
                                  ones_f[0:1, :].to_broadcast([1, HS]))
            onesP = cp.tile([1, P], R)
            nc.vector.tensor_copy(onesP[:],
                                  ones_f[0:1, :].to_broadcast([1, P]))
            epsv = cp.tile([P, 1], F32)
            nc.vector.memset(epsv[:], LN_EPS)
            identR = cp.tile([P, P], R)
            nc.vector.tensor_copy(identR[:], ident[:])

            # x load first so the big DMA isn't stuck behind the
            # scattered little bias loads
            xsb = bp.tile([P, NT, D], F32, tag="at", name="xsb")
            xr3 = x_d.rearrange("(t p) d -> p t d", p=P)
            for tb in range(NT):
                nc.sync.dma_start(xsb[:, tb, :], xr3[:, tb, :])

            bqv = cp.tile([P, EBS], F32)
            nc.sync.dma_start(bqv[:], bq_d.rearrange("h s -> (h s)")
                              .rearrange("(b p) -> p b", p=P))
            bkv = cp.tile([P, EBS], F32)
            nc.sync.dma_start(bkv[:], bk_d.rearrange("h s -> (h s)")
                              .rearrange("(b p) -> p b", p=P))
            bvv = cp.tile([P, EBS], F32)
            nc.sync.dma_start(bvv[:], bv_d.rearrange("h s -> (h s)")
                              .rearrange("(b p) -> p b", p=P))
            g1v = cp.tile([P, DB], F32)
            nc.sync.dma_start(g1v[:], g1_d.rearrange("(b p) -> p b", p=P))
            c1v = cp.tile([P, DB], F32)
            nc.sync.dma_start(c1v[:], c1_d.rearrange("(b p) -> p b", p=P))
            g2v = cp.tile([P, DB], F32)
            nc.sync.dma_start(g2v[:], g2_d.rearrange("(b p) -> p b", p=P))
            c2v = cp.tile([P, DB], F32)
            nc.sync.dma_start(c2v[:], c2_d.rearrange("(b p) -> p b", p=P))
            b1v = cp.tile([P, FF // P], F32)
            nc.sync.dma_start(b1v[:], b1_d.rearrange("(b p) -> p b", p=P))

            # LN stats scratch (reused for LN2 by tag)
            st_sum = cp.tile([P, NT], F32)
            st_sq = cp.tile([P, NT], F32)
            st_mu = cp.tile([P, NT], F32)
            st_var = cp.tile([P, NT], F32)
            st_rs = cp.tile([P, NT], F32)
            st_nm = cp.tile([P, NT], F32)
            st_vh = cp.tile([P, NT], F32)
            st_t = cp.tile([P, NT], F32)
            st_ih = cp.tile([P, NT], mybir.dt.int32)

            def layernorm_transpose(src, dst, gv, cv, pfx, pspool, trbufs,
                                    after_tb=None, tbs=None):
                """src: [P, NT, D] token layout (f32) -> dst [P, DB, N] f32r
                feature layout, with affine (gv, cv per-partition) folded into
                the transpose evacuation. Fully per-tb so each token tile
                flows stats -> normalize -> transpose independently."""
                for tb in (range(NT) if tbs is None else tbs):
                    t1 = (tb, tb + 1)
                    nc.vector.reduce_sum(st_sum[:, t1[0]:t1[1]], src[:, tb, :],
                                         axis=mybir.AxisListType.X)
                    sq = bp.tile([P, D], F32, tag="qb", bufs=2,
                                 name=f"sq{tb}")
                    nc.scalar.activation(sq[:], src[:, tb, :], AF.Square,
                                         accum_out=st_sq[:, t1[0]:t1[1]])
                    sm = st_sum[:, t1[0]:t1[1]]
                    var = st_var[:, t1[0]:t1[1]]
                    rs = st_rs[:, t1[0]:t1[1]]
                    nm = st_nm[:, t1[0]:t1[1]]
                    ih = st_ih[:, t1[0]:t1[1]]
                    vh = st_vh[:, t1[0]:t1[1]]
                    tt = st_t[:, t1[0]:t1[1]]
                    i32 = mybir.dt.int32
                    # var = sq/D - (sum/D)^2 + eps   (depth-3 chain)
                    nc.vector.tensor_tensor(tt, sm, sm, OP.mult)
                    nc.vector.tensor_scalar(var, tt, -1.0 / (D * D), LN_EPS,
                                            OP.mult, OP.add)
                    nc.vector.tensor_scalar(var, st_sq[:, t1[0]:t1[1]],
                                            1.0 / D, var, OP.mult, OP.add)
                    # rstd = rsqrt(var), DVE-only (bit hack + 2 Newton steps)
                    # so the ACT engine never needs the sqrt table set
                    nc.vector.tensor_scalar(ih, var.bitcast(i32), 1, None,
                                            OP.arith_shift_right)
                    nc.vector.tensor_scalar(rs.bitcast(i32), ih, -1,
                                            0x5F3759DF, OP.mult, OP.add)
                    nc.vector.tensor_scalar_mul(vh, var, -0.5)
                    for _ in range(2):
                        nc.vector.tensor_tensor(tt, rs, rs, OP.mult)
                        nc.vector.tensor_scalar(tt, tt, vh, 1.5,
                                                OP.mult, OP.add)
                        nc.vector.tensor_tensor(rs, rs, tt, OP.mult)
                    # nm = -(sum/D)*rstd
                    nc.vector.tensor_tensor(nm, sm, rs, OP.mult)
                    nc.vector.tensor_scalar_mul(nm, nm, -1.0 / D)
                    tnorm = bp.tile([P, D], F32, tag="kb", bufs=2,
                                    name=f"tn{tb}")
                    nc.vector.tensor_scalar(tnorm[:], src[:, tb, :],
                                            rs, nm, OP.mult, OP.add)
                    for db in range(DB):
                        pt = pspool.tile([P, P], F32, tag="tr", bufs=trbufs,
                                         name=f"ptr{pfx}_{tb}_{db}")
                        nc.tensor.transpose(pt[:], tnorm[:, ts(db, P)],
                                            ident[:])
                        nc.vector.tensor_scalar(dst[:, db, ts(tb, P)], pt[:],
                                                gv[:, db:db + 1],
                                                cv[:, db:db + 1],
                                                OP.mult, OP.add)
                    if after_tb is not None:
                        after_tb(tb)

            # ================= Phase A: LN1 + transpose ====================
            HT = bp.tile([P, DB, N], R, tag="ht", name="HT")
            psAB_cm = tc.tile_pool(name="psAB", bufs=1, space="PSUM")
            psAB = psAB_cm.__enter__()
            layernorm_transpose(xsb, HT, g1v, c1v, "a", psAB, 4)

            # ================= Phase B0: V projection ======================
            Vaug = bp.tile([P, NT, H, HS + 1], R, tag="v", name="Vaug")
            nc.vector.tensor_copy(
                Vaug[:, :, :, HS:HS + 1],
                ones_f[:, None, :].to_broadcast([P, NT, H, 1]))
            if True:
                for eh in range(2):
                    wvt = bp.tile([P, DB, 512], R, tag="se", bufs=2,
                                  name=f"wv{eh}")
                    for do in range(DB):
                        nc.sync.dma_start(
                            wvt[:, do].rearrange("p (h s) -> p h s", s=HS),
                            wv_d[eh * 8:(eh + 1) * 8, ds(do * P, P), :]
                            .rearrange("h dp s -> dp h s")
                            .bitcast(R))
                    for tb in range(NT):
                        pv = psAB.tile([P, 512], F32, tag="qkv", bufs=4,
                                       name=f"pv{eh}_{tb}")
                        for db in range(DB):
                            nc.tensor.matmul(pv[:], HT[:, db, ts(tb, P)],
                                             wvt[:, db, :],
                                             start=(db == 0), stop=(db == DB - 1))
                        nc.scalar.activation(
                            Vaug[:, tb, eh * 8:(eh + 1) * 8, 0:HS],
                            pv[:].rearrange("p (h s) -> p h s", s=HS), AF.Copy)

            # ============ Phase BC: Q/K per e-block fused with attention ===
            psAB_cm.__exit__(None, None, None)
            psBC_cm = tc.tile_pool(name="psBC", bufs=1, space="PSUM")
            psBC = psBC_cm.__enter__()
            attnT = bp.tile([P, EBS, N], R, tag="at", name="attnT")
            # prefetch Wproj during attention (4 x 1MB quarter-tiles)
            wp4 = []
            for g4 in range(4):
                wpt = bp.tile([P, 2, D], R, tag="sh", bufs=4, name=f"wp{g4}")
                nc.sync.dma_start(
                    wpt[:], wp_d[ds(g4 * 256, 256)]
                    .rearrange("(eo ep) d -> ep eo d", ep=P)
                    .bitcast(R))
                wp4.append(wpt)

            if True:
                for eb in range(EBS):
                    wqt = bp.tile([P, DB, P], R, tag="wqk", bufs=2,
                                  name=f"wq{eb}")
                    for do in range(DB):
                        nc.sync.dma_start(
                            wqt[:, do].rearrange("p (h s) -> p h s", s=HS),
                            wq_d[2 * eb:2 * eb + 2, ds(do * P, P), :]
                            .rearrange("h dp s -> dp h s")
                            .bitcast(R))
                    wkt = bp.tile([P, DB, P], R, tag="wqk", bufs=2,
                                  name=f"wk{eb}")
                    for do in range(DB):
                        nc.sync.dma_start(
                            wkt[:, do].rearrange("p (h s) -> p h s", s=HS),
                            wk_d[2 * eb:2 * eb + 2, ds(do * P, P), :]
                            .rearrange("h dp s -> dp h s")
                            .bitcast(R))
                    Qb = bp.tile([P, N], R, tag="qb", bufs=2, name=f"Qb{eb}")
                    Kb = bp.tile([P, N], R, tag="kb", bufs=2, name=f"Kb{eb}")
                    for nh in range(NH):
                        pq = psBC.tile([P, 512], F32, tag="qk", bufs=2,
                                       name=f"pq{eb}_{nh}")
                        for db in range(DB):
                            nc.tensor.matmul(pq[:], wqt[:, db, :],
                                             HT[:, db, ds(nh * 512, 512)],
                                             start=(db == 0), stop=(db == DB - 1))
                        nc.vector.tensor_scalar_add(Qb[:, ds(nh * 512, 512)],
                                                    pq[:], bqv[:, eb:eb + 1])
                        pk = psBC.tile([P, 512], F32, tag="qk", bufs=2,
                                       name=f"pk{eb}_{nh}")
                        for db in range(DB):
                            nc.tensor.matmul(pk[:], wkt[:, db, :],
                                             HT[:, db, ds(nh * 512, 512)],
                                             start=(db == 0), stop=(db == DB - 1))
                        nc.vector.tensor_scalar_add(Kb[:, ds(nh * 512, 512)],
                                                    pk[:], bkv[:, eb:eb + 1])

                    # attention for heads 2eb (partitions 0:64) and
                    # 2eb+1 (partitions 64:128), per n-half of 512
                    for nh in range(NH):
                        pts = [bp.tile([P, NT, 512], R, tag="se", bufs=2,
                                       name=f"PT{eb}_{nh}_{i}")
                               for i in range(2)]
                        # scoresT[m, n] = sum_s K[m,s] Q[n,s]; exp via ACT
                        for mt in range(NT):
                            for i in range(2):
                                base = i * HS
                                pss = psBC.tile([P, 512], F32, tag="sc",
                                                bufs=4, name=f"ps{eb}{nh}{mt}{i}")
                                nc.tensor.matmul(
                                    pss[:],
                                    Kb[base:base + HS, ts(mt, P)],
                                    Qb[base:base + HS, ds(nh * 512, 512)],
                                    start=True, stop=True)
                                nc.scalar.activation(pts[i][:, mt, :], pss[:],
                                                     AF.Exp, scale=0.125)
                        pas = [psBC.tile([HS + 1, 512], F32, tag="at65",
                                         bufs=2, name=f"pa{eb}_{nh}_{i}")
                               for i in range(2)]
                        for mb in range(NT):
                            for i in range(2):
                                nc.tensor.matmul(pas[i][:],
                                                 Vaug[:, mb, 2 * eb + i, :],
                                                 pts[i][:, mb, :],
                                                 start=(mb == 0),
                                                 stop=(mb == NT - 1))
                        for i in range(2):
                            base = i * HS
                            rec = bp.tile([1, 512], F32, tag="rb", bufs=2,
                                          name=f"rc{eb}_{nh}_{i}")
                            nc.vector.reciprocal(rec[:],
                                                 pas[i][HS:HS + 1, :])
                            rbs = bp.tile([HS, 512], F32, tag="rb", bufs=2,
                                          name=f"rb{eb}_{nh}_{i}")
                            nc.gpsimd.partition_broadcast(rbs[:], rec[:])
                            dstA = attnT[base:base + HS, eb,
                                         ds(nh * 512, 512)]
                            nc.vector.tensor_tensor(dstA, pas[i][0:HS, :],
                                                    rbs[:], OP.mult)
                            nc.vector.tensor_scalar_add(
                                dstA, dstA, bvv[base:base + HS, eb:eb + 1])

            # w1(ft0) prefetch into "se" (frees at end of attention);
            # high priority so the DMA issues as soon as the slot frees
            w1pre = bp.tile([P, DB, 512], R, tag="se", bufs=2,
                            name="w1pre")
            with tc.high_priority():
                nc.sync.dma_start(
                    w1pre[:],
                    w1_d[:, ds(0, 512)]
                    .rearrange("(do dp) f -> dp do f", dp=P)
                    .bitcast(R))

            # ================= Phase D: proj + residual ====================
            psBC_cm.__exit__(None, None, None)
            psDE_cm = tc.tile_pool(name="psDE", bufs=1, space="PSUM")
            psDE = psDE_cm.__enter__()
            x2 = bp.tile([P, NT, D], F32, tag="v", name="x2")
            xr = bp.tile([P, NT, D], R, tag="ht", name="xrl")
            if True:
                # broadcast bproj -> [P, D]
                bprow = bp.tile([1, D], R, tag="kb", bufs=2, name="bprow")
                nc.sync.dma_start(bprow[:], bp_d[None, :].bitcast(R))
                bpB = bp.tile([P, D], F32, tag="qb", bufs=2, name="bpB")
                for dh in range(2):
                    pbb = psDE.tile([P, 512], F32, tag="trb", bufs=1,
                                    name=f"pbb{dh}")
                    nc.tensor.matmul(pbb[:], onesP[:],
                                     bprow[:, ds(dh * 512, 512)],
                                     start=True, stop=True)
                    nc.vector.tensor_copy(bpB[:, ds(dh * 512, 512)], pbb[:])
                for tb in range(NT):
                    nc.sync.dma_start(xr[:, tb, :],
                                      xr3[:, tb, :].bitcast(R))
                    nc.vector.tensor_tensor(xr[:, tb, :], xr[:, tb, :],
                                            bpB[:], OP.add)
                for tb in range(NT):
                    for dt in range(2):
                        pp = psDE.tile([P, 512], F32, tag="pj", bufs=4,
                                       name=f"pp{tb}_{dt}")
                        for g4 in range(4):
                            for eo in range(2):
                                nc.tensor.matmul(
                                    pp[:], attnT[:, g4 * 2 + eo, ts(tb, P)],
                                    wp4[g4][:, eo, ds(dt * 512, 512)],
                                    start=(g4 == 0 and eo == 0),
                                    stop=False)
                        # residual folded into the PE accumulation
                        nc.tensor.matmul(pp[:], identR[:],
                                         xr[:, tb, ds(dt * 512, 512)],
                                         start=False, stop=True)
                        nc.scalar.activation(x2[:, tb, ds(dt * 512, 512)],
                                             pp[:], AF.Copy)

            # ================= Phase E: LN2, transpose, stash x2+b2 ========
            H2T = bp.tile([P, DB, N], R, tag="ht", name="H2T")
            if True:
                def ffn1_group(nt, ft, fc, _unused, pool, ptag, pbufs,
                               ydst, w1t, w1o):
                    p1 = pool.tile([P, 512], F32, tag=ptag, bufs=pbufs,
                                   name=f"p1_{nt}_{ft}_{fc}")
                    for db in range(DB):
                        nc.tensor.matmul(
                            p1[:], w1t[:, db, ds(w1o, P)],
                            H2T[:, db, ds(nt * 512, 512)],
                            start=(db == 0), stop=(db == DB - 1))
                    bf = ft * 4 + fc
                    if use_lrelu:
                        nc.scalar.activation(ydst, p1[:], AF.Prelu,
                                             bias=b1v[:, bf:bf + 1],
                                             alpha=0.01)
                    else:
                        z = bp.tile([P, 512], F32, tag="qb", bufs=2,
                                    name=f"z{nt}_{bf}")
                        nc.scalar.activation(z[:], p1[:], AF.Identity,
                                             bias=b1v[:, bf:bf + 1])
                        zs = bp.tile([P, 512], F32, tag="rb", bufs=2,
                                     name=f"zs{nt}_{bf}")
                        nc.vector.tensor_scalar_mul(zs[:], z[:], 0.01)
                        nc.vector.tensor_tensor(ydst, z[:], zs[:], OP.max)

                layernorm_transpose(x2, H2T, g2v, c2v, "e", psDE, 3)
                # broadcast b2 -> [P, D]; x2 += b2B; stash to DRAM
                b2row = bp.tile([1, D], R, tag="kb", bufs=2, name="b2row")
                nc.sync.dma_start(b2row[:], b2_d[None, :].bitcast(R))
                b2B = bp.tile([P, D], F32, tag="qb", bufs=2, name="b2B")
                for dh in range(2):
                    pb2 = psDE.tile([P, 512], F32, tag="trb", bufs=1,
                                    name=f"pb2{dh}")
                    nc.tensor.matmul(pb2[:], onesP[:],
                                     b2row[:, ds(dh * 512, 512)],
                                     start=True, stop=True)
                    nc.vector.tensor_copy(b2B[:, ds(dh * 512, 512)], pb2[:])
                for tb in range(NT):
                    nc.vector.tensor_tensor(x2[:, tb, :], x2[:, tb, :],
                                            b2B[:], OP.add)
                    nc.sync.dma_start(x2pb_d[:, tb, :], x2[:, tb, :])

            # ================= Phase F: FFN ================================
            psDE_cm.__exit__(None, None, None)
            psF_cm = tc.tile_pool(name="psF", bufs=1, space="PSUM")
            psF = psF_cm.__enter__()
            if True:
                for nt in range(NH):
                    y1 = [bp.tile([P, 16, 512], R, tag=tg,
                                  name=f"y1{nt}{tg}")
                          for tg in ("at", "v")]
                    xcf = bp.tile([P, 4, D], F32, tag="se", bufs=2,
                                  name=f"xcf{nt}")
                    nc.sync.dma_start(xcf[:], x2pb_d[:, nt * 4:(nt + 1) * 4, :])
                    for ft in range(FF // 512):
                        if ft == 0:
                            w1h = [w1pre, w1pre]
                            w1off = [0, 256]
                        else:
                            w1h = []
                            w1off = [0, 0]
                            for hh in range(2):
                                w1t = bp.tile([P, DB, 256], R, tag="sh",
                                              bufs=4, name=f"w1_{nt}_{ft}_{hh}")
                                nc.sync.dma_start(
                                    w1t[:],
                                    w1_d[:, ds(ft * 512 + hh * 256, 256)]
                                    .rearrange("(do dp) f -> dp do f", dp=P)
                                    .bitcast(R))
                                w1h.append(w1t)
                        for fc in range(4):
                            bf = ft * 4 + fc
                            ffn1_group(nt, ft, fc, None, psF, "fp", 8,
                                       y1[bf // 16][:, bf % 16, :],
                                       w1h[fc // 2],
                                       w1off[fc // 2] + (fc % 2) * P)
                    pf2 = [psF.tile([P, 512], F32, tag="fp", bufs=8,
                                    name=f"p2_{nt}_{j}") for j in range(8)]
                    NFT = FF // 512
                    def w2_halves(nt, ft):
                        hs = []
                        for hh in range(2):
                            w2t = bp.tile([P, 2, D], R, tag="sh", bufs=4,
                                          name=f"w2_{nt}_{ft}_{hh}")
                            nc.sync.dma_start(
                                w2t[:],
                                w2_d[ds(ft * 512 + hh * 256, 256), :]
                                .rearrange("(fo fp) d -> fp fo d", fp=P)
                                .bitcast(R))
                            hs.append(w2t)
                        return hs
                    for ft in range(NFT - 1):
                        w2h = w2_halves(nt, ft)
                        for fc in range(4):
                            bf = ft * 4 + fc
                            ysrc = y1[bf // 16][:, bf % 16, :]
                            for tb in range(4):
                                for dt in range(2):
                                    nc.tensor.matmul(
                                        pf2[tb * 2 + dt][:],
                                        ysrc[:, ts(tb, P)],
                                        w2h[fc // 2][:, fc % 2,
                                                     ds(dt * 512, 512)],
                                        start=(ft == 0 and fc == 0),
                                        stop=False)
                    # last f-tile: close each psum group in turn so its evac
                    # and output DMA overlap the remaining groups' matmuls
                    ftl = NFT - 1
                    w2h = w2_halves(nt, ftl)
                    for tb in range(4):
                        for dt in range(2):
                            for fc in range(4):
                                bf = ftl * 4 + fc
                                ysrc = y1[bf // 16][:, bf % 16, :]
                                nc.tensor.matmul(
                                    pf2[tb * 2 + dt][:],
                                    ysrc[:, ts(tb, P)],
                                    w2h[fc // 2][:, fc % 2,
                                                 ds(dt * 512, 512)],
                                    start=False, stop=(fc == 3))
                            rows = ds(nt * 512 + tb * P, P)
                            og = bp.tile([P, 512], F32, tag="rb", bufs=2,
                                         name=f"og{nt}_{tb}_{dt}")
                            nc.vector.tensor_tensor(og[:], pf2[tb * 2 + dt][:],
                                                    xcf[:, tb, ds(dt * 512, 512)],
                                                    OP.add)
                            nc.sync.dma_start(out_d[rows, ds(dt * 512, 512)],
                                              og[:])
            psF_cm.__exit__(None, None, None)
    nc.compile()
    return nc


def get_nc():
    global _CACHED_NC
    if _CACHED_NC is None:
        _CACHED_NC = build_nc()
    return _CACHED_NC


def kernel(**inputs):
    nc = get_nc()
    x = np.ascontiguousarray(np.asarray(inputs["x"], dtype=np.float32))
    B = x.shape[0]
    weights = {k: np.ascontiguousarray(np.asarray(v, dtype=np.float32))
               for k, v in inputs.items() if k != "x"}
    in_maps = [dict(weights, x=x[b]) for b in range(B)]
    res = run_bass_kernel_spmd(nc, in_maps, list(range(B)))
    return np.stack([res.results[b]["out"] for b in range(B)], axis=0)



# revision 2
# speedup vs baseline: 1.0058x; 1.0058x over previous
"""Trainium2 Bass kernel v2 for the pre-LN transformer block (MHA + FFN).

Data-parallel over batch: 8 NeuronCores, one batch element each.
Attention side runs fp8 e4m3 with DoubleRow matmuls (2 k-tiles/instr at
0.5 cy/row): QKV projections, scores (folded 32-partition layout),
attn@V (+ softmax denominator via a ones-column DoubleRow matmul), and
the output projection. The FFN runs bf16 (fp8 there busts the 2e-2
accuracy gate). Softmax exp is computed once on ACT with a 1/4 output
scale folded in (cancels in normalization) so probs fit fp8 range.
Work is software-pipelined over 4 token quarters so the ACT-bound exp
phase overlaps the PE-bound FFN of the previous quarter.

Weights are pre-quantized and laid out host-side in kernel().
"""
import sys

for _p in ("/opt/trn_rl_repo", "/root/.axon_site/_ro/trn_rl_repo"):
    if _p not in sys.path:
        sys.path.insert(0, _p)

import numpy as np
import ml_dtypes
import concourse.bass as bass
import concourse.tile as tile
from concourse import bacc, mybir
from concourse.bass import ds, ts
from concourse.bass_utils import run_bass_kernel_spmd
from concourse.masks import make_identity

P = 128
N = 1024
D = 1024
H = 16
HS = 64
FF = 4096
NT = 8            # token tiles of 128
DB = 8            # d-blocks of 128
NQ = 4            # pipeline quarters over tokens
QW = N // NQ      # 256 tokens per quarter
FC = FF // P      # 32 f-chunks
LN_EPS = 1e-5
MLN4 = -1.3862943611198906  # ln(1/4): exp output scale, cancels in softmax

F32 = mybir.dt.float32
R = mybir.dt.float32r
BF = mybir.dt.bfloat16
F8 = mybir.dt.float8e4
AF = mybir.ActivationFunctionType
OP = mybir.AluOpType
DR = mybir.MatmulPerfMode.DoubleRow

_CACHED_NC = None


def build_nc(use_lrelu=True):
    nc = bacc.Bacc("TRN2", target_bir_lowering=False, debug=False, num_devices=8)

    x_d = nc.dram_tensor("x", [N, D], F32, kind="ExternalInput").ap()
    wq_d = nc.dram_tensor("wq8", [P, DB, 4, 2, P], F8, kind="ExternalInput").ap()
    wk_d = nc.dram_tensor("wk8", [P, DB, 4, 2, P], F8, kind="ExternalInput").ap()
    wv_d = nc.dram_tensor("wv8", [P, DB, H * HS], F8, kind="ExternalInput").ap()
    wp_d = nc.dram_tensor("wp8", [P, DB, D], F8, kind="ExternalInput").ap()
    w1_d = nc.dram_tensor("w1h", [P, FC, DB, P], BF, kind="ExternalInput").ap()
    w2_d = nc.dram_tensor("w2h", [P, FC, D], BF, kind="ExternalInput").ap()
    bq_d = nc.dram_tensor("bqf", [P, 4, 2], F32, kind="ExternalInput").ap()
    bk_d = nc.dram_tensor("bkf", [P, 4, 2], F32, kind="ExternalInput").ap()
    bvpb_d = nc.dram_tensor("bvpb", [D], F32, kind="ExternalInput").ap()
    b1_d = nc.dram_tensor("b1f", [P, FC], F32, kind="ExternalInput").ap()
    b2_d = nc.dram_tensor("b2", [D], F32, kind="ExternalInput").ap()
    g1_d = nc.dram_tensor("ln1_g", [D], F32, kind="ExternalInput").ap()
    c1_d = nc.dram_tensor("ln1_b", [D], F32, kind="ExternalInput").ap()
    g2_d = nc.dram_tensor("ln2_g", [D], F32, kind="ExternalInput").ap()
    c2_d = nc.dram_tensor("ln2_b", [D], F32, kind="ExternalInput").ap()
    out_d = nc.dram_tensor("out", [N, D], F32, kind="ExternalOutput").ap()

    xr3 = x_d.rearrange("(t p) d -> p t d", p=P)
    outr3 = out_d.rearrange("(t p) d -> p t d", p=P)

    with tile.TileContext(nc) as tc:
        with tc.tile_pool(name="cn", bufs=1) as cp, \
             tc.tile_pool(name="big", bufs=1) as bp:
            # ---------------- constants / small vectors ----------------
            identB = cp.tile([P, P], BF)
            make_identity(nc, identB[:])
            onesPc = cp.tile([P, 1], F32)
            nc.vector.memset(onesPc[:], 1.0)
            onesF = bp.tile([1, QW], F32, tag="rc", bufs=2, name="onesF")
            nc.vector.memset(onesF[:], 1.0)
            onesRow = cp.tile([1, P], R)
            nc.vector.tensor_copy(onesRow[:], onesF[:, 0:P])
            onesRowR = onesRow
            mln4 = cp.tile([P, 1], F32)
            nc.vector.memset(mln4[:], MLN4)

            xsb = bp.tile([P, NT, D], F32, tag="xsb", name="xsb")
            for tb in range(NT):
                nc.sync.dma_start(xsb[:, tb, :], xr3[:, tb, :])

            g1v = cp.tile([P, DB], F32)
            nc.sync.dma_start(g1v[:], g1_d.rearrange("(b p) -> p b", p=P))
            c1v = cp.tile([P, DB], F32)
            nc.sync.dma_start(c1v[:], c1_d.rearrange("(b p) -> p b", p=P))
            g2v = cp.tile([P, DB], F32)
            nc.sync.dma_start(g2v[:], g2_d.rearrange("(b p) -> p b", p=P))
            c2v = cp.tile([P, DB], F32)
            nc.sync.dma_start(c2v[:], c2_d.rearrange("(b p) -> p b", p=P))
            bqv = cp.tile([P, 4, 2], F32)
            nc.sync.dma_start(bqv[:], bq_d)
            bkv = cp.tile([P, 4, 2], F32)
            nc.sync.dma_start(bkv[:], bk_d)
            b1v = cp.tile([P, FC], F32)
            nc.sync.dma_start(b1v[:], b1_d)
            b2row = cp.tile([1, D], R)
            nc.sync.dma_start(b2row[:], b2_d[None, :].bitcast(R))
            bvp = cp.tile([1, D], R)       # bv @ Wproj + bproj (host)
            nc.sync.dma_start(bvp[:], bvpb_d[None, :].bitcast(R))

            # fp8 weights (loaded once; W1/W2 bf16 streamed in chunks)
            wq8s = bp.tile([P, DB, 4, 2, P], F8, tag="wqk", bufs=2, name="wq8s")
            nc.sync.dma_start(wq8s[:], wq_d)
            wk8s = bp.tile([P, DB, 4, 2, P], F8, tag="wqk", bufs=2, name="wk8s")
            nc.sync.dma_start(wk8s[:], wk_d)
            wv8s = bp.tile([P, DB, H * HS], F8, tag="y1", bufs=2, name="wv8s")
            nc.sync.dma_start(wv8s[:], wv_d)
            wp8s = bp.tile([P, DB, D], F8, tag="wp", name="wp8s")
            nc.sync.dma_start(wp8s[:], wp_d)

            # LN stats scratch
            st_sum = cp.tile([P, NT], F32)
            st_sq = cp.tile([P, NT], F32)
            st_var = cp.tile([P, NT], F32)
            st_rs = cp.tile([P, NT], F32)
            st_nm = cp.tile([P, NT], F32)
            st_vh = cp.tile([P, NT], F32)
            st_t = cp.tile([P, NT], F32)
            st_ih = cp.tile([P, NT], mybir.dt.int32)

            def ln_reduce(src, tb, pfx):
                t1 = (tb, tb + 1)
                nc.vector.reduce_sum(st_sum[:, t1[0]:t1[1]], src,
                                     axis=mybir.AxisListType.X)
                dump = bp.tile([P, D], BF, tag="tn", bufs=2,
                               name=f"dmp{pfx}{tb}")
                nc.scalar.activation(dump[:], src, AF.Square,
                                     accum_out=st_sq[:, t1[0]:t1[1]])

            def ln_chain(lo, hi):
                """Vectorized var/rsqrt chain over st[:, lo:hi] (DVE-only
                rsqrt: bit hack + 2 Newton steps). Fills st_rs, st_nm."""
                sm = st_sum[:, lo:hi]
                sq = st_sq[:, lo:hi]
                var = st_var[:, lo:hi]
                rs = st_rs[:, lo:hi]
                nm = st_nm[:, lo:hi]
                ih = st_ih[:, lo:hi]
                vh = st_vh[:, lo:hi]
                tt = st_t[:, lo:hi]
                i32 = mybir.dt.int32
                # var = sq/D - (sum/D)^2 + eps
                nc.vector.tensor_tensor(tt, sm, sm, OP.mult)
                nc.vector.tensor_scalar(var, tt, -1.0 / (D * D), LN_EPS,
                                        OP.mult, OP.add)
                nc.vector.tensor_scalar_mul(tt, sq, 1.0 / D)
                nc.vector.tensor_tensor(var, var, tt, OP.add)
                nc.vector.tensor_scalar(ih, var.bitcast(i32), 1, None,
                                        OP.arith_shift_right)
                nc.vector.tensor_scalar(rs.bitcast(i32), ih, -1,
                                        0x5F3759DF, OP.mult, OP.add)
                nc.vector.tensor_scalar_mul(vh, var, -0.5)
                for _ in range(2):
                    nc.vector.tensor_tensor(tt, rs, rs, OP.mult)
                    nc.vector.tensor_tensor(tt, tt, vh, OP.mult)
                    nc.vector.tensor_scalar_add(tt, tt, 1.5)
                    nc.vector.tensor_tensor(rs, rs, tt, OP.mult)
                nc.vector.tensor_tensor(nm, sm, rs, OP.mult)
                nc.vector.tensor_scalar_mul(nm, nm, -1.0 / D)

            # ================= Phase A+B: LN1 -> HT8, QKV (fp8 DR) =========
            HT8 = bp.tile([P, DB, N], F8, tag="ht", name="HT8")
            Qf8 = bp.tile([P, 4, 2, N], F8, tag="qf", name="Qf8")
            Kf8 = bp.tile([P, 4, 2, N], F8, tag="kf", name="Kf8")
            V8 = bp.tile([P, NT, H, HS + 1], F8, tag="v8", name="V8")
            nc.vector.tensor_copy(
                V8[:, :, :, HS:HS + 1],
                onesPc[:, None, :].to_broadcast([P, NT, H, 1]))
            psB_cm = tc.tile_pool(name="psB", bufs=1, space="PSUM")
            psB = psB_cm.__enter__()

            def emit_ln1_half(half):
                trt = []
                for g in range(4):
                    pt = psB.tile([P, 512], F32, tag="mm", bufs=4,
                                  name=f"l1t{half}_{g}")
                    trt.append(pt.bitcast(BF).rearrange(
                        "p (a b c) -> p a b c", a=2, b=4))
                if half == 0:
                    ln_reduce(xsb[:, 0, :], 0, "a")
                    ln_chain(0, 1)
                    for tq in range(1, 4):
                        ln_reduce(xsb[:, tq, :], tq, "a")
                    ln_chain(1, 4)
                else:
                    for tq in range(4):
                        ln_reduce(xsb[:, 4 + tq, :], 4 + tq, "a")
                    ln_chain(4, 8)
                for tq in range(4):
                    tb = half * 4 + tq
                    tn = bp.tile([P, D], BF, tag="tn", bufs=2, name=f"tn{tb}")
                    nc.scalar.activation(tn[:], xsb[:, tb, :], AF.Identity,
                                         bias=st_nm[:, tb:tb + 1],
                                         scale=st_rs[:, tb:tb + 1])
                    for db in range(DB):
                        nc.tensor.transpose(trt[db // 2][:, db % 2, tq, :],
                                            tn[:, ts(db, P)], identB[:])
                for db in range(DB):
                    nc.vector.tensor_scalar(
                        HT8[:, db, ds(half * 512, 512)],
                        trt[db // 2][:, db % 2, :, :].rearrange(
                            "p a b -> p (a b)"),
                        g1v[:, db:db + 1], c1v[:, db:db + 1],
                        OP.mult, OP.add)

            def emit_v_block(tb, fh, pool, tagbufs):
                p = pool.tile([P, 512], F32, tag=tagbufs[0], bufs=tagbufs[1],
                              name=f"pv{tb}_{fh}")
                for j in range(4):
                    nc.tensor.matmul(
                        p[:], HT8[:, 2 * j:2 * j + 2, ts(tb, P)],
                        wv8s[:, 2 * j:2 * j + 2, ds(fh * 512, 512)],
                        start=(j == 0), stop=(j == 3), perf_mode=DR)
                nc.vector.tensor_copy(
                    V8[:, tb, 8 * fh:8 * fh + 8, 0:HS],
                    p.rearrange("p (h s) -> p h s", s=HS))

            def emit_qk_group(wsrc, bsrc, dstf, pfx, t, lh, nh2):
                p = psB.tile([P, 512], F32, tag="mm", bufs=4,
                             name=f"p{pfx}{t}{lh}{nh2}")
                for j in range(4):
                    nc.tensor.matmul(
                        p[:], wsrc[:, 2 * j:2 * j + 2, t, lh, :],
                        HT8[:, 2 * j:2 * j + 2, ds(nh2 * 512, 512)],
                        start=(j == 0), stop=(j == 3), perf_mode=DR)
                if pfx == "q":
                    nc.scalar.activation(
                        dstf[:, t, lh, ds(nh2 * 512, 512)], p[:],
                        AF.Identity, bias=bsrc[:, t, lh:lh + 1])
                else:
                    nc.vector.tensor_scalar_add(
                        dstf[:, t, lh, ds(nh2 * 512, 512)], p[:],
                        bsrc[:, t, lh:lh + 1])

            # LN1 half0 -> half0-token matmuls -> LN1 half1 -> rest
            for nh2 in range(2):
                emit_ln1_half(nh2)
                for tb in range(4 * nh2, 4 * nh2 + 4):
                    emit_v_block(tb, 0, psB, ("mm", 4))
                for (wsrc, bsrc, dstf, pfx) in ((wq8s, bqv, Qf8, "q"),
                                                (wk8s, bkv, Kf8, "k")):
                    for t in range(4):
                        for lh in range(2):
                            emit_qk_group(wsrc, bsrc, dstf, pfx, t, lh, nh2)

            bvpR = bvp
            b2rowR = b2row
            psB_cm.__exit__(None, None, None)

            # ============== Steady state: per-quarter pipeline =============
            attnN8 = bp.tile([P, DB, N], F8, tag="ht", name="attnN8")
            x2 = bp.tile([P, NT, D], F32, tag="x2", name="x2")
            H2T16 = bp.tile([P, DB, N], BF, tag="h2t", name="H2T16")

            psS_cm = tc.tile_pool(name="psS", bufs=1, space="PSUM")
            psS = psS_cm.__enter__()

            def emit_scores_exp(h, qr, ptsq):
                tb0, ntb = qr
                w = ntb * P
                t, i = h // 4, h % 4
                ip = ds(32 * i, 32)
                for jp in range(2):
                    psc = psS.tile([P, 4, QW], F32, tag="sc", bufs=2,
                                   name=f"sc{h}_{tb0}_{jp}")
                    for u in range(4):
                        mt = 4 * jp + u
                        nc.tensor.matmul(
                            psc[:, u, 0:w],
                            Kf8[ip, t, :, ts(mt, P)],
                            Qf8[ip, t, :, ds(tb0 * P, w)],
                            start=True, stop=True, perf_mode=DR,
                            tile_position=(32 * i, 0))
                    nc.scalar.activation(ptsq[:, 4 * jp:4 * jp + 4, 0:w],
                                         psc[:, :, 0:w], AF.Exp,
                                         bias=mln4[:], scale=0.125)

            def emit_attnv(h, qr, ptsq):
                tb0, ntb = qr
                w = ntb * P
                pavt = psS.tile([P, 512], F32, tag="mm", bufs=2,
                                name=f"av{h}_{tb0}")
                pav = pavt[:, 0:w]
                for j in range(4):
                    sl = ptsq[:, 2 * j:2 * j + 2, 0:w]
                    nc.tensor.matmul(pav[0:65, :],
                                     V8[:, 2 * j:2 * j + 2, h, :],
                                     sl, start=(j == 0), stop=(j == 3),
                                     perf_mode=DR)
                rec = bp.tile([1, QW], F32, tag="rc", bufs=2,
                              name=f"rc{h}_{tb0}")
                nc.vector.reciprocal(rec[:, 0:w], pav[64:65, :])
                rbs = bp.tile([64, QW], F32, tag="rb", bufs=2,
                              name=f"rb{h}_{tb0}")
                nc.gpsimd.partition_broadcast(rbs[:, 0:w], rec[:, 0:w])
                nc.vector.tensor_tensor(
                    attnN8[ds(64 * (h % 2), 64), h // 2, ds(tb0 * P, w)],
                    pav[0:64, :], rbs[:, 0:w], OP.mult)

            def emit_proj(qr):
                tb0, ntb = qr
                for tq in range(ntb):
                    tb = tb0 + tq
                    for dt in range(2):
                        p = psS.tile([P, 512], F32, tag="mm", bufs=2,
                                     name=f"pp{tb}_{dt}")
                        for j in range(4):
                            nc.tensor.matmul(
                                p[:], attnN8[:, 2 * j:2 * j + 2, ts(tb, P)],
                                wp8s[:, 2 * j:2 * j + 2, ds(dt * 512, 512)],
                                start=(j == 0), stop=False, perf_mode=DR)
                        # rank-1 (bv@Wp + bproj) row folded into the group
                        nc.tensor.matmul(p[:], onesRowR[:],
                                         bvpR[:, ds(dt * 512, 512)],
                                         start=False, stop=True)
                        nc.vector.tensor_tensor(
                            x2[:, tb, ds(dt * 512, 512)], p[:],
                            xsb[:, tb, ds(dt * 512, 512)], OP.add)

            def emit_ln2(qr):
                tb0, ntb = qr
                trt = []
                for g in range(2):
                    pt = psS.tile([P, 512], F32, tag="mm", bufs=2,
                                  name=f"l2t{tb0}_{g}")
                    trt.append(pt.bitcast(BF).rearrange(
                        "p (a b c) -> p a b c", a=4, b=2))
                for tq in range(ntb):
                    ln_reduce(x2[:, tb0 + tq, :], tb0 + tq, "e")
                ln_chain(tb0, tb0 + ntb)
                for tq in range(ntb):
                    tb = tb0 + tq
                    tn = bp.tile([P, D], BF, tag="tn", bufs=2,
                                 name=f"t2{tb0}_{tq}")
                    nc.scalar.activation(tn[:], x2[:, tb, :], AF.Identity,
                                         bias=st_nm[:, tb:tb + 1],
                                         scale=st_rs[:, tb:tb + 1])
                    for db in range(DB):
                        nc.tensor.transpose(trt[db // 4][:, db % 4, tq, :],
                                            tn[:, ts(db, P)], identB[:])
                for db in range(DB):
                    nc.vector.tensor_scalar(
                        H2T16[:, db, ds(tb0 * P, ntb * P)],
                        trt[db // 4][:, db % 4, 0:ntb, :].rearrange(
                            "p a b -> p (a b)"),
                        g2v[:, db:db + 1], c2v[:, db:db + 1],
                        OP.mult, OP.add)

            w1cache = {}

            def emit_ffn1_chunk(qr, fc, y1q):
                tb0, ntb = qr
                w = ntb * P
                if fc % 2 == 0:
                    w1c = bp.tile([P, 2, DB, P], BF, tag="w1c", bufs=2,
                                  name=f"w1c{tb0}_{fc}")
                    nc.gpsimd.dma_start(w1c[:], w1_d[:, fc:fc + 2])
                    w1cache[0] = w1c
                w1c = w1cache[0]
                p = psS.tile([P, 512], F32, tag="mm", bufs=2,
                             name=f"f1{tb0}_{fc}")
                for db in range(DB):
                    nc.tensor.matmul(p[:, 0:w], w1c[:, fc % 2, db, :],
                                     H2T16[:, db, ds(tb0 * P, w)],
                                     start=(db == 0), stop=(db == DB - 1))
                if use_lrelu:
                    nc.scalar.activation(y1q[:, fc, 0:w], p[:, 0:w], AF.Prelu,
                                         bias=b1v[:, fc:fc + 1], alpha=0.01)
                else:
                    z = bp.tile([P, QW], F32, tag="zz", bufs=2,
                                name=f"z{tb0}_{fc}")
                    nc.scalar.activation(z[:, 0:w], p[:, 0:w], AF.Identity,
                                         bias=b1v[:, fc:fc + 1])
                    zs = bp.tile([P, QW], F32, tag="rb", bufs=2,
                                 name=f"zs{tb0}_{fc}")
                    nc.vector.tensor_scalar_mul(zs[:, 0:w], z[:, 0:w], 0.01)
                    nc.vector.tensor_tensor(y1q[:, fc, 0:w], z[:, 0:w],
                                            zs[:, 0:w], OP.max)

            def emit_ffn2_dt(qr, y1q, dt):
                tb0, ntb = qr
                fbw = 2
                if True:
                    pf = [psS.tile([P, 512], F32, tag="f2", bufs=2,
                                   name=f"f2{tb0}_{dt}_{tq}")
                          for tq in range(ntb)]
                    for fbp in range(FC // fbw):
                        w2c = bp.tile([P, fbw, 512], BF, tag="w2c", bufs=3,
                                      name=f"w2c{tb0}_{dt}_{fbp}")
                        nc.sync.dma_start(
                            w2c[:],
                            w2_d[:, fbw * fbp:fbw * fbp + fbw,
                                 ds(dt * 512, 512)])
                        for u in range(fbw):
                            fb = fbw * fbp + u
                            for tq in range(ntb):
                                nc.tensor.matmul(pf[tq][:],
                                                 y1q[:, fb, ts(tq, P)],
                                                 w2c[:, u, :],
                                                 start=(fb == 0), stop=False)
                    for tq in range(ntb):
                        tb = tb0 + tq
                        nc.tensor.matmul(pf[tq][:], onesRowR[:],
                                         b2rowR[:, ds(dt * 512, 512)],
                                         start=False, stop=True)
                        og = bp.tile([P, 512], F32, tag="og", bufs=2,
                                     name=f"og{tb0}_{dt}_{tq}")
                        nc.vector.tensor_tensor(og[:], pf[tq][:],
                                                x2[:, tb, ds(dt * 512, 512)],
                                                OP.add)
                        nc.gpsimd.dma_start(outr3[:, tb, ds(dt * 512, 512)],
                                            og[:])

            # pipeline: h-loop(q) interleaves FFN1(q-1); then FFN2(q-1),
            # proj(q), LN2(q).
            y1_prev = None
            QUARTERS = [(0, 2), (2, 2), (4, 2), (6, 2)]
            for qi, qr in enumerate(QUARTERS):
                prev = None
                for h in range(H):
                    ptsq = bp.tile([P, NT, QW], F8, tag="wqk", bufs=2,
                                   name=f"pts{qr[0]}_{h}")
                    emit_scores_exp(h, qr, ptsq)
                    if y1_prev is not None:
                        emit_ffn1_chunk(y1_prev[0], 2 * h, y1_prev[1])
                    elif qi == 0 and h < 2:
                        # all fh=1 V blocks must precede the first attnV
                        # evac (attnN8 reuses HT8's region)
                        for tb4 in range(4 * h, 4 * h + 4):
                            emit_v_block(tb4, 1, psS, ("mm", 2))
                    if prev is not None:
                        emit_attnv(prev[0], qr, prev[1])
                    if y1_prev is not None:
                        emit_ffn1_chunk(y1_prev[0], 2 * h + 1, y1_prev[1])
                    prev = (h, ptsq)
                emit_attnv(prev[0], qr, prev[1])
                if y1_prev is not None:
                    emit_ffn2_dt(y1_prev[0], y1_prev[1], 0)
                emit_proj(qr)
                if y1_prev is not None:
                    emit_ffn2_dt(y1_prev[0], y1_prev[1], 1)
                emit_ln2(qr)
                y1_prev = (qr, bp.tile([P, FC, QW], BF, tag="y1", bufs=2,
                                       name=f"y1_{qr[0]}"))
            # drain: FFN for the last quarter
            for fc in range(FC):
                emit_ffn1_chunk(y1_prev[0], fc, y1_prev[1])
            emit_ffn2_dt(y1_prev[0], y1_prev[1], 0)
            emit_ffn2_dt(y1_prev[0], y1_prev[1], 1)
            psS_cm.__exit__(None, None, None)
    nc.compile()
    return nc


def get_nc():
    global _CACHED_NC
    if _CACHED_NC is None:
        _CACHED_NC = build_nc()
    return _CACHED_NC


def _prep_weights(inputs):
    """Host-side layout + quantization (shared across cores)."""
    f32 = np.float32
    E4 = ml_dtypes.float8_e4m3
    BF16 = ml_dtypes.bfloat16
    q8 = lambda a: np.ascontiguousarray(a).astype(E4)
    qb = lambda a: np.ascontiguousarray(a).astype(BF16)
    g = lambda k: np.asarray(inputs[k], dtype=f32)

    Wq, Wk, Wv = g("Wq"), g("Wk"), g("Wv")
    Wp, W1, W2 = g("Wproj"), g("W1"), g("W2")
    bq, bk, bv = g("bq"), g("bk"), g("bv")

    def fold_qk(W):  # [H, D, HS] -> [dp, db, t, lh, i*32+hs32]
        a = W.reshape(4, 4, D // P, P, 2, 32)      # t i db dp lh h32
        return q8(a.transpose(3, 2, 0, 4, 1, 5).reshape(P, DB, 4, 2, P))

    def fold_bqk(b):  # [H, HS] -> [p=i*32+h32, t, lh]
        a = b.reshape(4, 4, 2, 32)                 # t i lh h32
        return np.ascontiguousarray(
            a.transpose(1, 3, 0, 2).reshape(P, 4, 2), dtype=f32)

    return {
        "wq8": fold_qk(Wq),
        "wk8": fold_qk(Wk),
        "wv8": q8(Wv.reshape(H, DB, P, HS).transpose(2, 1, 0, 3)
                  .reshape(P, DB, H * HS)),
        "wp8": q8(Wp.reshape(DB, P, D).transpose(1, 0, 2)),
        "w1h": qb(W1.reshape(DB, P, FC, P).transpose(1, 2, 0, 3)),
        "w2h": qb(W2.reshape(FC, P, D).transpose(1, 0, 2)),
        "bqf": fold_bqk(bq),
        "bkf": fold_bqk(bk),
        "bvpb": (bv.reshape(H * HS).astype(np.float64)
                 @ Wp.astype(np.float64)).astype(f32) + g("bproj"),
        "b1f": np.ascontiguousarray(g("b1").reshape(FC, P).T, dtype=f32),
        "b2": g("b2"),
        "ln1_g": g("ln1_g"), "ln1_b": g("ln1_b"),
        "ln2_g": g("ln2_g"), "ln2_b": g("ln2_b"),
    }


def kernel(**inputs):
    nc = get_nc()
    x = np.ascontiguousarray(np.asarray(inputs["x"], dtype=np.float32))
    B = x.shape[0]
    shared = _prep_weights(inputs)
    in_maps = [dict(shared, x=np.ascontiguousarray(x[b])) for b in range(B)]
    res = run_bass_kernel_spmd(nc, in_maps, list(range(B)))
    return np.stack([res.results[b]["out"] for b in range(B)], axis=0)


# revision 3
# speedup vs baseline: 1.0192x; 1.0133x over previous
"""Trainium2 Bass kernel v2 for the pre-LN transformer block (MHA + FFN).

Data-parallel over batch: 8 NeuronCores, one batch element each.
Attention side runs fp8 e4m3 with DoubleRow matmuls (2 k-tiles/instr at
0.5 cy/row): QKV projections, scores (folded 32-partition layout),
attn@V (+ softmax denominator via a ones-column DoubleRow matmul), and
the output projection. The FFN runs bf16 (fp8 there busts the 2e-2
accuracy gate). Softmax exp is computed once on ACT with a 1/4 output
scale folded in (cancels in normalization) so probs fit fp8 range.
Work is software-pipelined over 4 token quarters so the ACT-bound exp
phase overlaps the PE-bound FFN of the previous quarter.

Weights are pre-quantized and laid out host-side in kernel().
"""
import sys

for _p in ("/opt/trn_rl_repo", "/root/.axon_site/_ro/trn_rl_repo"):
    if _p not in sys.path:
        sys.path.insert(0, _p)

import numpy as np
import ml_dtypes
import concourse.bass as bass
import concourse.tile as tile
from concourse import bacc, mybir
from concourse.bass import ds, ts
from concourse.bass_utils import run_bass_kernel_spmd
from concourse.masks import make_identity

P = 128
N = 1024
D = 1024
H = 16
HS = 64
FF = 4096
NT = 8            # token tiles of 128
DB = 8            # d-blocks of 128
NQ = 4            # pipeline quarters over tokens
QW = N // NQ      # 256 tokens per quarter
FC = FF // P      # 32 f-chunks
LN_EPS = 1e-5
MLN4 = -1.3862943611198906  # ln(1/4): exp output scale, cancels in softmax

F32 = mybir.dt.float32
R = mybir.dt.float32r
BF = mybir.dt.bfloat16
F8 = mybir.dt.float8e4
AF = mybir.ActivationFunctionType
OP = mybir.AluOpType
DR = mybir.MatmulPerfMode.DoubleRow

_CACHED_NC = None


def build_nc(use_lrelu=True):
    nc = bacc.Bacc("TRN2", target_bir_lowering=False, debug=False, num_devices=8)

    x_d = nc.dram_tensor("x", [N, D], F32, kind="ExternalInput").ap()
    wq_d = nc.dram_tensor("wq8", [P, DB, 4, 2, P], F8, kind="ExternalInput").ap()
    wk_d = nc.dram_tensor("wk8", [P, DB, 4, 2, P], F8, kind="ExternalInput").ap()
    wv_d = nc.dram_tensor("wv8", [P, DB, H * HS], F8, kind="ExternalInput").ap()
    wp_d = nc.dram_tensor("wp8", [P, DB, D], F8, kind="ExternalInput").ap()
    w1_d = nc.dram_tensor("w1h", [P, FC, DB, P], BF, kind="ExternalInput").ap()
    w2_d = nc.dram_tensor("w2h", [P, FC, D], BF, kind="ExternalInput").ap()
    bq_d = nc.dram_tensor("bqf", [P, 4, 2], F32, kind="ExternalInput").ap()
    bk_d = nc.dram_tensor("bkf", [P, 4, 2], F32, kind="ExternalInput").ap()
    bvpb_d = nc.dram_tensor("bvpb", [D], F32, kind="ExternalInput").ap()
    b1_d = nc.dram_tensor("b1f", [P, FC], F32, kind="ExternalInput").ap()
    b2_d = nc.dram_tensor("b2", [D], F32, kind="ExternalInput").ap()
    g1_d = nc.dram_tensor("ln1_g", [D], F32, kind="ExternalInput").ap()
    c1_d = nc.dram_tensor("ln1_b", [D], F32, kind="ExternalInput").ap()
    g2_d = nc.dram_tensor("ln2_g", [D], F32, kind="ExternalInput").ap()
    c2_d = nc.dram_tensor("ln2_b", [D], F32, kind="ExternalInput").ap()
    out_d = nc.dram_tensor("out", [N, D], F32, kind="ExternalOutput").ap()

    xr3 = x_d.rearrange("(t p) d -> p t d", p=P)
    outr3 = out_d.rearrange("(t p) d -> p t d", p=P)

    with tile.TileContext(nc) as tc:
        with tc.tile_pool(name="cn", bufs=1) as cp, \
             tc.tile_pool(name="big", bufs=1) as bp:
            # ---------------- constants / small vectors ----------------
            identB = cp.tile([P, P], BF)
            make_identity(nc, identB[:])
            onesPc = cp.tile([P, 1], F32)
            nc.vector.memset(onesPc[:], 1.0)
            onesF = bp.tile([1, QW], F32, tag="rc", bufs=2, name="onesF")
            nc.vector.memset(onesF[:], 1.0)
            onesRow = cp.tile([1, P], R)
            nc.vector.tensor_copy(onesRow[:], onesF[:, 0:P])
            onesRowR = onesRow
            mln4 = cp.tile([P, 1], F32)
            nc.vector.memset(mln4[:], MLN4)

            xsb = bp.tile([P, NT, D], F32, tag="xsb", name="xsb")
            wq8s = bp.tile([P, DB, 4, 2, P], F8, tag="wqk", bufs=2, name="wq8s")
            wk8s = bp.tile([P, DB, 4, 2, P], F8, tag="wqk", bufs=2, name="wk8s")
            wv8s = bp.tile([P, DB, H * HS], F8, tag="y1", bufs=2, name="wv8s")
            wp8s = bp.tile([P, DB, D], F8, tag="wp", name="wp8s")
            for tb in range(NT):
                nc.sync.dma_start(xsb[:, tb, :], xr3[:, tb, :])
            g1v = cp.tile([P, DB], F32)
            nc.sync.dma_start(g1v[:], g1_d.rearrange("(b p) -> p b", p=P))
            c1v = cp.tile([P, DB], F32)
            nc.sync.dma_start(c1v[:], c1_d.rearrange("(b p) -> p b", p=P))
            g2v = cp.tile([P, DB], F32)
            nc.sync.dma_start(g2v[:], g2_d.rearrange("(b p) -> p b", p=P))
            c2v = cp.tile([P, DB], F32)
            nc.sync.dma_start(c2v[:], c2_d.rearrange("(b p) -> p b", p=P))
            bqv = cp.tile([P, 4, 2], F32)
            nc.sync.dma_start(bqv[:], bq_d)
            bkv = cp.tile([P, 4, 2], F32)
            nc.sync.dma_start(bkv[:], bk_d)
            b1v = cp.tile([P, FC], F32)
            nc.sync.dma_start(b1v[:], b1_d)
            b2row = cp.tile([1, D], R)
            nc.sync.dma_start(b2row[:], b2_d[None, :].bitcast(R))
            bvp = cp.tile([1, D], R)       # bv @ Wproj + bproj (host)
            nc.sync.dma_start(bvp[:], bvpb_d[None, :].bitcast(R))
            nc.sync.dma_start(wq8s[:], wq_d)
            nc.sync.dma_start(wk8s[:], wk_d)
            nc.sync.dma_start(wv8s[:], wv_d)
            nc.sync.dma_start(wp8s[:], wp_d)

            # LN stats scratch
            st_sum = cp.tile([P, NT], F32)
            st_sq = cp.tile([P, NT], F32)
            st_var = cp.tile([P, NT], F32)
            st_rs = cp.tile([P, NT], F32)
            st_nm = cp.tile([P, NT], F32)
            st_vh = cp.tile([P, NT], F32)
            st_t = cp.tile([P, NT], F32)
            st_ih = cp.tile([P, NT], mybir.dt.int32)

            def ln_reduce(src, tb, pfx):
                t1 = (tb, tb + 1)
                nc.vector.reduce_sum(st_sum[:, t1[0]:t1[1]], src,
                                     axis=mybir.AxisListType.X)
                dump = bp.tile([P, D], BF, tag="tn", bufs=2,
                               name=f"dmp{pfx}{tb}")
                nc.scalar.activation(dump[:], src, AF.Square,
                                     accum_out=st_sq[:, t1[0]:t1[1]])

            def ln_chain(lo, hi):
                """Vectorized var/rsqrt chain over st[:, lo:hi] (DVE-only
                rsqrt: bit hack + 2 Newton steps). Fills st_rs, st_nm."""
                sm = st_sum[:, lo:hi]
                sq = st_sq[:, lo:hi]
                var = st_var[:, lo:hi]
                rs = st_rs[:, lo:hi]
                nm = st_nm[:, lo:hi]
                ih = st_ih[:, lo:hi]
                vh = st_vh[:, lo:hi]
                tt = st_t[:, lo:hi]
                i32 = mybir.dt.int32
                # var = sq/D - (sum/D)^2 + eps
                nc.vector.tensor_tensor(tt, sm, sm, OP.mult)
                nc.vector.tensor_scalar(var, tt, -1.0 / (D * D), LN_EPS,
                                        OP.mult, OP.add)
                nc.vector.tensor_scalar_mul(tt, sq, 1.0 / D)
                nc.vector.tensor_tensor(var, var, tt, OP.add)
                nc.vector.tensor_scalar(ih, var.bitcast(i32), 1, None,
                                        OP.arith_shift_right)
                nc.vector.tensor_scalar(rs.bitcast(i32), ih, -1,
                                        0x5F3759DF, OP.mult, OP.add)
                nc.vector.tensor_scalar_mul(vh, var, -0.5)
                for _ in range(2):
                    nc.vector.tensor_tensor(tt, rs, rs, OP.mult)
                    nc.vector.tensor_tensor(tt, tt, vh, OP.mult)
                    nc.vector.tensor_scalar_add(tt, tt, 1.5)
                    nc.vector.tensor_tensor(rs, rs, tt, OP.mult)
                nc.vector.tensor_tensor(nm, sm, rs, OP.mult)
                nc.vector.tensor_scalar_mul(nm, nm, -1.0 / D)

            # ================= Phase A+B: LN1 -> HT8, QKV (fp8 DR) =========
            HT8 = bp.tile([P, DB, N], F8, tag="ht", name="HT8")
            Qf8 = bp.tile([P, 4, 2, N], F8, tag="qf", name="Qf8")
            Kf8 = bp.tile([P, 4, 2, N], F8, tag="kf", name="Kf8")
            V8 = bp.tile([P, NT, H, HS + 1], F8, tag="v8", name="V8")
            nc.vector.tensor_copy(
                V8[:, :, :, HS:HS + 1],
                onesPc[:, None, :].to_broadcast([P, NT, H, 1]))
            psB_cm = tc.tile_pool(name="psB", bufs=1, space="PSUM")
            psB = psB_cm.__enter__()

            def emit_ln1_half(half):
                trt = []
                for g in range(4):
                    pt = psB.tile([P, 512], F32, tag="mm", bufs=4,
                                  name=f"l1t{half}_{g}")
                    trt.append(pt.bitcast(BF).rearrange(
                        "p (a b c) -> p a b c", a=2, b=4))
                if half == 0:
                    ln_reduce(xsb[:, 0, :], 0, "a")
                    ln_chain(0, 1)
                    for tq in range(1, 4):
                        ln_reduce(xsb[:, tq, :], tq, "a")
                    ln_chain(1, 4)
                else:
                    for tq in range(4):
                        ln_reduce(xsb[:, 4 + tq, :], 4 + tq, "a")
                    ln_chain(4, 8)
                for tq in range(4):
                    tb = half * 4 + tq
                    tn = bp.tile([P, D], BF, tag="tn", bufs=2, name=f"tn{tb}")
                    nc.scalar.activation(tn[:], xsb[:, tb, :], AF.Identity,
                                         bias=st_nm[:, tb:tb + 1],
                                         scale=st_rs[:, tb:tb + 1])
                    for db in range(DB):
                        nc.tensor.transpose(trt[db // 2][:, db % 2, tq, :],
                                            tn[:, ts(db, P)], identB[:])
                for db in range(DB):
                    nc.vector.tensor_scalar(
                        HT8[:, db, ds(half * 512, 512)],
                        trt[db // 2][:, db % 2, :, :].rearrange(
                            "p a b -> p (a b)"),
                        g1v[:, db:db + 1], c1v[:, db:db + 1],
                        OP.mult, OP.add)

            def emit_v_block(tb, fh, pool, tagbufs):
                p = pool.tile([P, 512], F32, tag=tagbufs[0], bufs=tagbufs[1],
                              name=f"pv{tb}_{fh}")
                for j in range(4):
                    nc.tensor.matmul(
                        p[:], HT8[:, 2 * j:2 * j + 2, ts(tb, P)],
                        wv8s[:, 2 * j:2 * j + 2, ds(fh * 512, 512)],
                        start=(j == 0), stop=(j == 3), perf_mode=DR)
                if fh == 0:   # phase B: DVE is loaded, use ACT
                    nc.scalar.copy(
                        V8[:, tb, 0:8, 0:HS],
                        p.rearrange("p (h s) -> p h s", s=HS))
                else:
                    nc.vector.tensor_copy(
                        V8[:, tb, 8:16, 0:HS],
                        p.rearrange("p (h s) -> p h s", s=HS))

            def emit_qk_group(wsrc, bsrc, dstf, pfx, t, lh, nh2):
                p = psB.tile([P, 512], F32, tag="mm", bufs=4,
                             name=f"p{pfx}{t}{lh}{nh2}")
                for j in range(4):
                    nc.tensor.matmul(
                        p[:], wsrc[:, 2 * j:2 * j + 2, t, lh, :],
                        HT8[:, 2 * j:2 * j + 2, ds(nh2 * 512, 512)],
                        start=(j == 0), stop=(j == 3), perf_mode=DR)
                if pfx == "q":
                    nc.scalar.activation(
                        dstf[:, t, lh, ds(nh2 * 512, 512)], p[:],
                        AF.Identity, bias=bsrc[:, t, lh:lh + 1])
                else:
                    nc.vector.tensor_scalar_add(
                        dstf[:, t, lh, ds(nh2 * 512, 512)], p[:],
                        bsrc[:, t, lh:lh + 1])

            # LN1 half0 -> half0-token matmuls -> LN1 half1 -> rest
            for nh2 in range(2):
                emit_ln1_half(nh2)
                for tb in range(4 * nh2, 4 * nh2 + 4):
                    emit_v_block(tb, 0, psB, ("mm", 4))
                for (wsrc, bsrc, dstf, pfx) in ((wq8s, bqv, Qf8, "q"),
                                                (wk8s, bkv, Kf8, "k")):
                    for t in range(4):
                        for lh in range(2):
                            emit_qk_group(wsrc, bsrc, dstf, pfx, t, lh, nh2)

            bvpR = bvp
            b2rowR = b2row
            psB_cm.__exit__(None, None, None)

            # ============== Steady state: per-quarter pipeline =============
            attnN8 = bp.tile([P, DB, N], F8, tag="ht", name="attnN8")
            x2 = bp.tile([P, NT, D], F32, tag="x2", name="x2")
            H2T16 = bp.tile([P, DB, N], BF, tag="h2t", name="H2T16")

            psS_cm = tc.tile_pool(name="psS", bufs=1, space="PSUM")
            psS = psS_cm.__enter__()

            def emit_scores_exp(h, qr, ptsq):
                tb0, ntb = qr
                w = ntb * P
                t, i = h // 4, h % 4
                ip = ds(32 * i, 32)
                for jp in range(2):
                    psc = psS.tile([P, 4, QW], F32, tag="sc", bufs=2,
                                   name=f"sc{h}_{tb0}_{jp}")
                    for u in range(4):
                        mt = 4 * jp + u
                        nc.tensor.matmul(
                            psc[:, u, 0:w],
                            Kf8[ip, t, :, ts(mt, P)],
                            Qf8[ip, t, :, ds(tb0 * P, w)],
                            start=True, stop=True, perf_mode=DR,
                            tile_position=(32 * i, 0))
                    nc.scalar.activation(ptsq[:, 4 * jp:4 * jp + 4, 0:w],
                                         psc[:, :, 0:w], AF.Exp,
                                         bias=mln4[:], scale=0.125)

            def emit_attnv(h, qr, ptsq):
                tb0, ntb = qr
                w = ntb * P
                pavt = psS.tile([P, 512], F32, tag="mm", bufs=2,
                                name=f"av{h}_{tb0}")
                pav = pavt[:, 0:w]
                for j in range(4):
                    sl = ptsq[:, 2 * j:2 * j + 2, 0:w]
                    nc.tensor.matmul(pav[0:65, :],
                                     V8[:, 2 * j:2 * j + 2, h, :],
                                     sl, start=(j == 0), stop=(j == 3),
                                     perf_mode=DR)
                rec = bp.tile([1, QW], F32, tag="rc", bufs=2,
                              name=f"rc{h}_{tb0}")
                nc.vector.reciprocal(rec[:, 0:w], pav[64:65, :])
                rbs = bp.tile([64, QW], F32, tag="rb", bufs=2,
                              name=f"rb{h}_{tb0}")
                nc.gpsimd.partition_broadcast(rbs[:, 0:w], rec[:, 0:w])
                nc.vector.tensor_tensor(
                    attnN8[ds(64 * (h % 2), 64), h // 2, ds(tb0 * P, w)],
                    pav[0:64, :], rbs[:, 0:w], OP.mult)

            def emit_proj(qr):
                tb0, ntb = qr
                for tq in range(ntb):
                    tb = tb0 + tq
                    for dt in range(2):
                        p = psS.tile([P, 512], F32, tag="mm", bufs=2,
                                     name=f"pp{tb}_{dt}")
                        for j in range(4):
                            nc.tensor.matmul(
                                p[:], attnN8[:, 2 * j:2 * j + 2, ts(tb, P)],
                                wp8s[:, 2 * j:2 * j + 2, ds(dt * 512, 512)],
                                start=(j == 0), stop=False, perf_mode=DR)
                        # rank-1 (bv@Wp + bproj) row folded into the group
                        nc.tensor.matmul(p[:], onesRowR[:],
                                         bvpR[:, ds(dt * 512, 512)],
                                         start=False, stop=True)
                        nc.vector.tensor_tensor(
                            x2[:, tb, ds(dt * 512, 512)], p[:],
                            xsb[:, tb, ds(dt * 512, 512)], OP.add)

            def emit_ln2(qr):
                tb0, ntb = qr
                trt = []
                for g in range(2):
                    pt = psS.tile([P, 512], F32, tag="mm", bufs=2,
                                  name=f"l2t{tb0}_{g}")
                    trt.append(pt.bitcast(BF).rearrange(
                        "p (a b c) -> p a b c", a=4, b=2))
                for tq in range(ntb):
                    ln_reduce(x2[:, tb0 + tq, :], tb0 + tq, "e")
                ln_chain(tb0, tb0 + ntb)
                for tq in range(ntb):
                    tb = tb0 + tq
                    tn = bp.tile([P, D], BF, tag="tn", bufs=2,
                                 name=f"t2{tb0}_{tq}")
                    nc.scalar.activation(tn[:], x2[:, tb, :], AF.Identity,
                                         bias=st_nm[:, tb:tb + 1],
                                         scale=st_rs[:, tb:tb + 1])
                    for db in range(DB):
                        nc.tensor.transpose(trt[db // 4][:, db % 4, tq, :],
                                            tn[:, ts(db, P)], identB[:])
                for db in range(DB):
                    nc.vector.tensor_scalar(
                        H2T16[:, db, ds(tb0 * P, ntb * P)],
                        trt[db // 4][:, db % 4, 0:ntb, :].rearrange(
                            "p a b -> p (a b)"),
                        g2v[:, db:db + 1], c2v[:, db:db + 1],
                        OP.mult, OP.add)

            w1cache = {}

            def emit_ffn1_chunk(qr, fc, y1q):
                tb0, ntb = qr
                w = ntb * P
                if fc % 2 == 0:
                    w1c = bp.tile([P, 2, DB, P], BF, tag="w1c", bufs=3,
                                  name=f"w1c{tb0}_{fc}")
                    nc.gpsimd.dma_start(w1c[:], w1_d[:, fc:fc + 2])
                    w1cache[0] = w1c
                w1c = w1cache[0]
                p = psS.tile([P, 512], F32, tag="mm", bufs=2,
                             name=f"f1{tb0}_{fc}")
                for db in range(DB):
                    nc.tensor.matmul(p[:, 0:w], w1c[:, fc % 2, db, :],
                                     H2T16[:, db, ds(tb0 * P, w)],
                                     start=(db == 0), stop=(db == DB - 1))
                if use_lrelu:
                    nc.scalar.activation(y1q[:, fc, 0:w], p[:, 0:w], AF.Prelu,
                                         bias=b1v[:, fc:fc + 1], alpha=0.01)
                else:
                    z = bp.tile([P, QW], F32, tag="zz", bufs=2,
                                name=f"z{tb0}_{fc}")
                    nc.scalar.activation(z[:, 0:w], p[:, 0:w], AF.Identity,
                                         bias=b1v[:, fc:fc + 1])
                    zs = bp.tile([P, QW], F32, tag="rb", bufs=2,
                                 name=f"zs{tb0}_{fc}")
                    nc.vector.tensor_scalar_mul(zs[:, 0:w], z[:, 0:w], 0.01)
                    nc.vector.tensor_tensor(y1q[:, fc, 0:w], z[:, 0:w],
                                            zs[:, 0:w], OP.max)

            def emit_ffn2_dt(qr, y1q, dt):
                tb0, ntb = qr
                fbw = 2
                if True:
                    pf = [psS.tile([P, 512], F32, tag="f2", bufs=2,
                                   name=f"f2{tb0}_{dt}_{tq}")
                          for tq in range(ntb)]
                    for fbp in range(FC // fbw):
                        w2c = bp.tile([P, fbw, 512], BF, tag="w2c", bufs=3,
                                      name=f"w2c{tb0}_{dt}_{fbp}")
                        nc.sync.dma_start(
                            w2c[:],
                            w2_d[:, fbw * fbp:fbw * fbp + fbw,
                                 ds(dt * 512, 512)])
                        for u in range(fbw):
                            fb = fbw * fbp + u
                            for tq in range(ntb):
                                nc.tensor.matmul(pf[tq][:],
                                                 y1q[:, fb, ts(tq, P)],
                                                 w2c[:, u, :],
                                                 start=(fb == 0), stop=False)
                    for tq in range(ntb):
                        tb = tb0 + tq
                        nc.tensor.matmul(pf[tq][:], onesRowR[:],
                                         b2rowR[:, ds(dt * 512, 512)],
                                         start=False, stop=True)
                        og = bp.tile([P, 512], F32, tag="og", bufs=2,
                                     name=f"og{tb0}_{dt}_{tq}")
                        nc.vector.tensor_tensor(og[:], pf[tq][:],
                                                x2[:, tb, ds(dt * 512, 512)],
                                                OP.add)
                        nc.gpsimd.dma_start(outr3[:, tb, ds(dt * 512, 512)],
                                            og[:])

            # pipeline: h-loop(q) interleaves FFN1(q-1); then FFN2(q-1),
            # proj(q), LN2(q).
            y1_prev = None
            QUARTERS = [(0, 2), (2, 2), (4, 2), (6, 2)]
            for qi, qr in enumerate(QUARTERS):
                prev = None
                for h in range(H):
                    ptsq = bp.tile([P, NT, QW], F8, tag="wqk", bufs=2,
                                   name=f"pts{qr[0]}_{h}")
                    emit_scores_exp(h, qr, ptsq)
                    if y1_prev is not None:
                        emit_ffn1_chunk(y1_prev[0], 2 * h, y1_prev[1])
                    elif qi == 0 and h < 2:
                        # all fh=1 V blocks must precede the first attnV
                        # evac (attnN8 reuses HT8's region)
                        for tb4 in range(4 * h, 4 * h + 4):
                            emit_v_block(tb4, 1, psS, ("mm", 2))
                    if prev is not None:
                        emit_attnv(prev[0], qr, prev[1])
                    if y1_prev is not None:
                        emit_ffn1_chunk(y1_prev[0], 2 * h + 1, y1_prev[1])
                    prev = (h, ptsq)
                emit_attnv(prev[0], qr, prev[1])
                if y1_prev is not None:
                    emit_ffn2_dt(y1_prev[0], y1_prev[1], 0)
                emit_proj(qr)
                if y1_prev is not None:
                    emit_ffn2_dt(y1_prev[0], y1_prev[1], 1)
                emit_ln2(qr)
                y1_prev = (qr, bp.tile([P, FC, QW], BF, tag="y1", bufs=2,
                                       name=f"y1_{qr[0]}"))
            # drain: FFN for the last quarter
            for fc in range(FC):
                emit_ffn1_chunk(y1_prev[0], fc, y1_prev[1])
            emit_ffn2_dt(y1_prev[0], y1_prev[1], 0)
            emit_ffn2_dt(y1_prev[0], y1_prev[1], 1)
            psS_cm.__exit__(None, None, None)
    nc.compile()
    return nc


def get_nc():
    global _CACHED_NC
    if _CACHED_NC is None:
        _CACHED_NC = build_nc()
    return _CACHED_NC


def _prep_weights(inputs):
    """Host-side layout + quantization (shared across cores)."""
    f32 = np.float32
    E4 = ml_dtypes.float8_e4m3
    BF16 = ml_dtypes.bfloat16
    q8 = lambda a: np.ascontiguousarray(a).astype(E4)
    qb = lambda a: np.ascontiguousarray(a).astype(BF16)
    g = lambda k: np.asarray(inputs[k], dtype=f32)

    Wq, Wk, Wv = g("Wq"), g("Wk"), g("Wv")
    Wp, W1, W2 = g("Wproj"), g("W1"), g("W2")
    bq, bk, bv = g("bq"), g("bk"), g("bv")

    def fold_qk(W):  # [H, D, HS] -> [dp, db, t, lh, i*32+hs32]
        a = W.reshape(4, 4, D // P, P, 2, 32)      # t i db dp lh h32
        return q8(a.transpose(3, 2, 0, 4, 1, 5).reshape(P, DB, 4, 2, P))

    def fold_bqk(b):  # [H, HS] -> [p=i*32+h32, t, lh]
        a = b.reshape(4, 4, 2, 32)                 # t i lh h32
        return np.ascontiguousarray(
            a.transpose(1, 3, 0, 2).reshape(P, 4, 2), dtype=f32)

    return {
        "wq8": fold_qk(Wq),
        "wk8": fold_qk(Wk),
        "wv8": q8(Wv.reshape(H, DB, P, HS).transpose(2, 1, 0, 3)
                  .reshape(P, DB, H * HS)),
        "wp8": q8(Wp.reshape(DB, P, D).transpose(1, 0, 2)),
        "w1h": qb(W1.reshape(DB, P, FC, P).transpose(1, 2, 0, 3)),
        "w2h": qb(W2.reshape(FC, P, D).transpose(1, 0, 2)),
        "bqf": fold_bqk(bq),
        "bkf": fold_bqk(bk),
        "bvpb": (bv.reshape(H * HS).astype(np.float64)
                 @ Wp.astype(np.float64)).astype(f32) + g("bproj"),
        "b1f": np.ascontiguousarray(g("b1").reshape(FC, P).T, dtype=f32),
        "b2": g("b2"),
        "ln1_g": g("ln1_g"), "ln1_b": g("ln1_b"),
        "ln2_g": g("ln2_g"), "ln2_b": g("ln2_b"),
    }


def kernel(**inputs):
    nc = get_nc()
    x = np.ascontiguousarray(np.asarray(inputs["x"], dtype=np.float32))
    B = x.shape[0]
    shared = _prep_weights(inputs)
    in_maps = [dict(shared, x=np.ascontiguousarray(x[b])) for b in range(B)]
    res = run_bass_kernel_spmd(nc, in_maps, list(range(B)))
    return np.stack([res.results[b]["out"] for b in range(B)], axis=0)


# revision 4
# speedup vs baseline: 1.0645x; 1.0444x over previous
"""Trainium2 Bass kernel v2 for the pre-LN transformer block (MHA + FFN).

Data-parallel over batch: 8 NeuronCores, one batch element each.
Attention side runs fp8 e4m3 with DoubleRow matmuls (2 k-tiles/instr at
0.5 cy/row): QKV projections, scores (folded 32-partition layout),
attn@V (+ softmax denominator via a ones-column DoubleRow matmul), and
the output projection. The FFN runs bf16 (fp8 there busts the 2e-2
accuracy gate). Softmax exp is computed once on ACT with a 1/4 output
scale folded in (cancels in normalization) so probs fit fp8 range.
Work is software-pipelined over 4 token quarters so the ACT-bound exp
phase overlaps the PE-bound FFN of the previous quarter.

Weights are pre-quantized and laid out host-side in kernel().
"""
import sys

for _p in ("/opt/trn_rl_repo", "/root/.axon_site/_ro/trn_rl_repo"):
    if _p not in sys.path:
        sys.path.insert(0, _p)

import numpy as np
import ml_dtypes
import concourse.bass as bass
import concourse.tile as tile
from concourse import bacc, mybir
from concourse.bass import ds, ts
from concourse.bass_utils import run_bass_kernel_spmd
from concourse.masks import make_identity

P = 128
N = 1024
D = 1024
H = 16
HS = 64
FF = 4096
NT = 8            # token tiles of 128
DB = 8            # d-blocks of 128
NQ = 4            # pipeline quarters over tokens
QW = N // NQ      # 256 tokens per quarter
FC = FF // P      # 32 f-chunks
LN_EPS = 1e-5
MLN4 = -1.3862943611198906  # ln(1/4): exp output scale, cancels in softmax

F32 = mybir.dt.float32
R = mybir.dt.float32r
BF = mybir.dt.bfloat16
F8 = mybir.dt.float8e4
AF = mybir.ActivationFunctionType
OP = mybir.AluOpType
DR = mybir.MatmulPerfMode.DoubleRow

_CACHED_NC = None


def build_nc(use_lrelu=True):
    nc = bacc.Bacc("TRN2", target_bir_lowering=False, debug=False, num_devices=8)

    x_d = nc.dram_tensor("x", [N, D], F32, kind="ExternalInput").ap()
    wq_d = nc.dram_tensor("wq8", [P, DB, 4, 2, P], F8, kind="ExternalInput").ap()
    wk_d = nc.dram_tensor("wk8", [P, DB, 4, 2, P], F8, kind="ExternalInput").ap()
    wv_d = nc.dram_tensor("wv8", [P, DB, H * HS], F8, kind="ExternalInput").ap()
    wp_d = nc.dram_tensor("wp8", [P, DB, D], F8, kind="ExternalInput").ap()
    w1_d = nc.dram_tensor("w1h", [P, FC, DB, P], BF, kind="ExternalInput").ap()
    w2_d = nc.dram_tensor("w2h", [P, FC, D], BF, kind="ExternalInput").ap()
    bq_d = nc.dram_tensor("bqf", [P, 4, 2], F32, kind="ExternalInput").ap()
    bk_d = nc.dram_tensor("bkf", [P, 4, 2], F32, kind="ExternalInput").ap()
    bvpb_d = nc.dram_tensor("bvpb", [D], F32, kind="ExternalInput").ap()
    b1_d = nc.dram_tensor("b1f", [P, FC], F32, kind="ExternalInput").ap()
    b2_d = nc.dram_tensor("b2", [D], F32, kind="ExternalInput").ap()
    g1_d = nc.dram_tensor("ln1_g", [D], F32, kind="ExternalInput").ap()
    c1_d = nc.dram_tensor("ln1_b", [D], F32, kind="ExternalInput").ap()
    g2_d = nc.dram_tensor("ln2_g", [D], F32, kind="ExternalInput").ap()
    c2_d = nc.dram_tensor("ln2_b", [D], F32, kind="ExternalInput").ap()
    out_d = nc.dram_tensor("out", [N, D], F32, kind="ExternalOutput").ap()

    xr3 = x_d.rearrange("(t p) d -> p t d", p=P)
    outr3 = out_d.rearrange("(t p) d -> p t d", p=P)

    with tile.TileContext(nc) as tc:
        with tc.tile_pool(name="cn", bufs=1) as cp, \
             tc.tile_pool(name="big", bufs=1) as bp:
            # ---------------- constants / small vectors ----------------
            identB = cp.tile([P, P], BF)
            make_identity(nc, identB[:])
            onesPc = cp.tile([P, 1], F32)
            nc.vector.memset(onesPc[:], 1.0)
            onesF = bp.tile([1, QW], F32, tag="rc", bufs=2, name="onesF")
            nc.vector.memset(onesF[:], 1.0)
            onesRow = cp.tile([1, P], R)
            nc.vector.tensor_copy(onesRow[:], onesF[:, 0:P])
            onesRowR = onesRow
            mln4 = cp.tile([P, 1], F32)
            nc.vector.memset(mln4[:], MLN4)

            xsb = bp.tile([P, NT, D], F32, tag="xsb", name="xsb")
            wq8s = bp.tile([P, DB, 4, 2, P], F8, tag="wqk", bufs=2, name="wq8s")
            wk8s = bp.tile([P, DB, 4, 2, P], F8, tag="wqk", bufs=2, name="wk8s")
            wv8s = bp.tile([P, DB, H * HS], F8, tag="y1", bufs=2, name="wv8s")
            wp8s = bp.tile([P, DB, D], F8, tag="wp", name="wp8s")
            for tb in range(NT):
                nc.sync.dma_start(xsb[:, tb, :], xr3[:, tb, :])
            g1v = cp.tile([P, DB], F32)
            nc.sync.dma_start(g1v[:], g1_d.rearrange("(b p) -> p b", p=P))
            c1v = cp.tile([P, DB], F32)
            nc.sync.dma_start(c1v[:], c1_d.rearrange("(b p) -> p b", p=P))
            g2v = cp.tile([P, DB], F32)
            nc.sync.dma_start(g2v[:], g2_d.rearrange("(b p) -> p b", p=P))
            c2v = cp.tile([P, DB], F32)
            nc.sync.dma_start(c2v[:], c2_d.rearrange("(b p) -> p b", p=P))
            bqv = cp.tile([P, 4, 2], F32)
            nc.sync.dma_start(bqv[:], bq_d)
            bkv = cp.tile([P, 4, 2], F32)
            nc.sync.dma_start(bkv[:], bk_d)
            b1v = cp.tile([P, FC], F32)
            nc.sync.dma_start(b1v[:], b1_d)
            b2row = cp.tile([1, D], R)
            nc.sync.dma_start(b2row[:], b2_d[None, :].bitcast(R))
            bvp = cp.tile([1, D], R)       # bv @ Wproj + bproj (host)
            nc.sync.dma_start(bvp[:], bvpb_d[None, :].bitcast(R))
            nc.sync.dma_start(wq8s[:], wq_d)
            nc.sync.dma_start(wk8s[:], wk_d)
            nc.sync.dma_start(wv8s[:], wv_d)
            nc.sync.dma_start(wp8s[:], wp_d)

            # LN stats scratch
            st_sum = cp.tile([P, NT], F32)
            st_sq = cp.tile([P, NT], F32)
            st_var = cp.tile([P, NT], F32)
            st_rs = cp.tile([P, NT], F32)
            st_nm = cp.tile([P, NT], F32)
            st_vh = cp.tile([P, NT], F32)
            st_t = cp.tile([P, NT], F32)
            st_ih = cp.tile([P, NT], mybir.dt.int32)

            def ln_reduce(src, tb, pfx):
                t1 = (tb, tb + 1)
                nc.vector.reduce_sum(st_sum[:, t1[0]:t1[1]], src,
                                     axis=mybir.AxisListType.X)
                dump = bp.tile([P, D], BF, tag="tn", bufs=2,
                               name=f"dmp{pfx}{tb}")
                nc.scalar.activation(dump[:], src, AF.Square,
                                     accum_out=st_sq[:, t1[0]:t1[1]])

            def ln_chain(lo, hi):
                """Vectorized var/rsqrt chain over st[:, lo:hi] (DVE-only
                rsqrt: bit hack + 2 Newton steps). Fills st_rs, st_nm."""
                sm = st_sum[:, lo:hi]
                sq = st_sq[:, lo:hi]
                var = st_var[:, lo:hi]
                rs = st_rs[:, lo:hi]
                nm = st_nm[:, lo:hi]
                ih = st_ih[:, lo:hi]
                vh = st_vh[:, lo:hi]
                tt = st_t[:, lo:hi]
                i32 = mybir.dt.int32
                # var = sq/D - (sum/D)^2 + eps
                nc.vector.tensor_tensor(tt, sm, sm, OP.mult)
                nc.vector.tensor_scalar(var, tt, -1.0 / (D * D), LN_EPS,
                                        OP.mult, OP.add)
                nc.vector.tensor_scalar_mul(tt, sq, 1.0 / D)
                nc.vector.tensor_tensor(var, var, tt, OP.add)
                nc.vector.tensor_scalar(ih, var.bitcast(i32), 1, None,
                                        OP.arith_shift_right)
                nc.vector.tensor_scalar(rs.bitcast(i32), ih, -1,
                                        0x5F3759DF, OP.mult, OP.add)
                nc.vector.tensor_scalar_mul(vh, var, -0.5)
                for _ in range(2):
                    nc.vector.tensor_tensor(tt, rs, rs, OP.mult)
                    nc.vector.tensor_tensor(tt, tt, vh, OP.mult)
                    nc.vector.tensor_scalar_add(tt, tt, 1.5)
                    nc.vector.tensor_tensor(rs, rs, tt, OP.mult)
                nc.vector.tensor_tensor(nm, sm, rs, OP.mult)
                nc.vector.tensor_scalar_mul(nm, nm, -1.0 / D)

            # ================= Phase A+B: LN1 -> HT8, QKV (fp8 DR) =========
            HT8 = bp.tile([P, DB, N], F8, tag="ht", name="HT8")
            Qf8 = bp.tile([P, 4, 2, N], F8, tag="qf", name="Qf8")
            Kf8 = bp.tile([P, 4, 2, N], F8, tag="kf", name="Kf8")
            V8 = bp.tile([P, NT, H, HS + 1], F8, tag="v8", name="V8")
            nc.vector.tensor_copy(
                V8[:, :, :, HS:HS + 1],
                onesPc[:, None, :].to_broadcast([P, NT, H, 1]))
            psB_cm = tc.tile_pool(name="psB", bufs=1, space="PSUM")
            psB = psB_cm.__enter__()

            def emit_ln1_half(half):
                trt = []
                for g in range(4):
                    pt = psB.tile([P, 512], F32, tag="mm", bufs=4,
                                  name=f"l1t{half}_{g}")
                    trt.append(pt.bitcast(BF).rearrange(
                        "p (a b c) -> p a b c", a=2, b=4))
                if half == 0:
                    ln_reduce(xsb[:, 0, :], 0, "a")
                    ln_chain(0, 1)
                    for tq in range(1, 4):
                        ln_reduce(xsb[:, tq, :], tq, "a")
                    ln_chain(1, 4)
                else:
                    for tq in range(4):
                        ln_reduce(xsb[:, 4 + tq, :], 4 + tq, "a")
                    ln_chain(4, 8)
                for tq in range(4):
                    tb = half * 4 + tq
                    tn = bp.tile([P, D], BF, tag="tn", bufs=2, name=f"tn{tb}")
                    nc.scalar.activation(tn[:], xsb[:, tb, :], AF.Identity,
                                         bias=st_nm[:, tb:tb + 1],
                                         scale=st_rs[:, tb:tb + 1])
                    for db in range(DB):
                        nc.tensor.transpose(trt[db // 2][:, db % 2, tq, :],
                                            tn[:, ts(db, P)], identB[:])
                for db in range(DB):
                    nc.vector.tensor_scalar(
                        HT8[:, db, ds(half * 512, 512)],
                        trt[db // 2][:, db % 2, :, :].rearrange(
                            "p a b -> p (a b)"),
                        g1v[:, db:db + 1], c1v[:, db:db + 1],
                        OP.mult, OP.add)

            def emit_v_block(tb, fh, pool, tagbufs):
                p = pool.tile([P, 512], F32, tag=tagbufs[0], bufs=tagbufs[1],
                              name=f"pv{tb}_{fh}")
                for j in range(4):
                    nc.tensor.matmul(
                        p[:], HT8[:, 2 * j:2 * j + 2, ts(tb, P)],
                        wv8s[:, 2 * j:2 * j + 2, ds(fh * 512, 512)],
                        start=(j == 0), stop=(j == 3), perf_mode=DR)
                if fh == 0:   # phase B: DVE is loaded, use ACT
                    nc.scalar.copy(
                        V8[:, tb, 0:8, 0:HS],
                        p.rearrange("p (h s) -> p h s", s=HS))
                else:
                    nc.vector.tensor_copy(
                        V8[:, tb, 8:16, 0:HS],
                        p.rearrange("p (h s) -> p h s", s=HS))

            def emit_qk_group(wsrc, bsrc, dstf, pfx, t, lh, nh2,
                              pool=None, bufs=4, on_act=True):
                pool = pool or psB
                p = pool.tile([P, 512], F32, tag="mm", bufs=bufs,
                              name=f"p{pfx}{t}{lh}{nh2}")
                for j in range(4):
                    nc.tensor.matmul(
                        p[:], wsrc[:, 2 * j:2 * j + 2, t, lh, :],
                        HT8[:, 2 * j:2 * j + 2, ds(nh2 * 512, 512)],
                        start=(j == 0), stop=(j == 3), perf_mode=DR)
                if on_act:
                    nc.scalar.activation(
                        dstf[:, t, lh, ds(nh2 * 512, 512)], p[:],
                        AF.Identity, bias=bsrc[:, t, lh:lh + 1])
                else:
                    nc.vector.tensor_scalar_add(
                        dstf[:, t, lh, ds(nh2 * 512, 512)], p[:],
                        bsrc[:, t, lh:lh + 1])

            # LN1 half0 -> half0-token matmuls -> LN1 half1 -> rest
            for nh2 in range(2):
                emit_ln1_half(nh2)
                for tb in range(4 * nh2, 4 * nh2 + 4):
                    emit_v_block(tb, 0, psB, ("mm", 4))
                for (wsrc, bsrc, dstf, pfx) in ((wq8s, bqv, Qf8, "q"),
                                                (wk8s, bkv, Kf8, "k")):
                    for t in range(4):
                        for lh in range(2):
                            emit_qk_group(wsrc, bsrc, dstf, pfx, t, lh, nh2,
                                          on_act=(pfx == "q"))

            bvpR = bvp
            b2rowR = b2row
            psB_cm.__exit__(None, None, None)

            # ============== Steady state: per-quarter pipeline =============
            attnN8 = bp.tile([P, DB, N], F8, tag="ht", name="attnN8")
            x2 = bp.tile([P, NT, D], F32, tag="x2", name="x2")
            H2T16 = bp.tile([P, DB, N], BF, tag="h2t", name="H2T16")

            psS_cm = tc.tile_pool(name="psS", bufs=1, space="PSUM")
            psS = psS_cm.__enter__()

            def emit_scores_exp(h, qr, ptsq):
                tb0, ntb = qr
                w = ntb * P
                t, i = h // 4, h % 4
                ip = ds(32 * i, 32)
                for jp in range(2):
                    psc = psS.tile([P, 4, QW], F32, tag="sc", bufs=2,
                                   name=f"sc{h}_{tb0}_{jp}")
                    for u in range(4):
                        mt = 4 * jp + u
                        nc.tensor.matmul(
                            psc[:, u, 0:w],
                            Kf8[ip, t, :, ts(mt, P)],
                            Qf8[ip, t, :, ds(tb0 * P, w)],
                            start=True, stop=True, perf_mode=DR,
                            tile_position=(32 * i, 0))
                    nc.scalar.activation(ptsq[:, 4 * jp:4 * jp + 4, 0:w],
                                         psc[:, :, 0:w], AF.Exp,
                                         bias=mln4[:], scale=0.125)

            def emit_attnv(h, qr, ptsq):
                tb0, ntb = qr
                w = ntb * P
                pavt = psS.tile([P, 512], F32, tag="mm", bufs=2,
                                name=f"av{h}_{tb0}")
                pav = pavt[:, 0:w]
                for j in range(4):
                    sl = ptsq[:, 2 * j:2 * j + 2, 0:w]
                    nc.tensor.matmul(pav[0:65, :],
                                     V8[:, 2 * j:2 * j + 2, h, :],
                                     sl, start=(j == 0), stop=(j == 3),
                                     perf_mode=DR)
                rec = bp.tile([1, QW], F32, tag="rc", bufs=2,
                              name=f"rc{h}_{tb0}")
                nc.vector.reciprocal(rec[:, 0:w], pav[64:65, :])
                rbs = bp.tile([64, QW], F32, tag="rb", bufs=2,
                              name=f"rb{h}_{tb0}")
                nc.gpsimd.partition_broadcast(rbs[:, 0:w], rec[:, 0:w])
                nc.vector.tensor_tensor(
                    attnN8[ds(64 * (h % 2), 64), h // 2, ds(tb0 * P, w)],
                    pav[0:64, :], rbs[:, 0:w], OP.mult)

            def emit_proj(qr):
                tb0, ntb = qr
                for tq in range(ntb):
                    tb = tb0 + tq
                    for dt in range(2):
                        p = psS.tile([P, 512], F32, tag="mm", bufs=2,
                                     name=f"pp{tb}_{dt}")
                        for j in range(4):
                            nc.tensor.matmul(
                                p[:], attnN8[:, 2 * j:2 * j + 2, ts(tb, P)],
                                wp8s[:, 2 * j:2 * j + 2, ds(dt * 512, 512)],
                                start=(j == 0), stop=False, perf_mode=DR)
                        # rank-1 (bv@Wp + bproj) row folded into the group
                        nc.tensor.matmul(p[:], onesRowR[:],
                                         bvpR[:, ds(dt * 512, 512)],
                                         start=False, stop=True)
                        nc.vector.tensor_tensor(
                            x2[:, tb, ds(dt * 512, 512)], p[:],
                            xsb[:, tb, ds(dt * 512, 512)], OP.add)

            def emit_ln2(qr):
                tb0, ntb = qr
                trt = []
                for g in range(2):
                    pt = psS.tile([P, 512], F32, tag="mm", bufs=2,
                                  name=f"l2t{tb0}_{g}")
                    trt.append(pt.bitcast(BF).rearrange(
                        "p (a b c) -> p a b c", a=4, b=2))
                for tq in range(ntb):
                    ln_reduce(x2[:, tb0 + tq, :], tb0 + tq, "e")
                ln_chain(tb0, tb0 + ntb)
                for tq in range(ntb):
                    tb = tb0 + tq
                    tn = bp.tile([P, D], BF, tag="tn", bufs=2,
                                 name=f"t2{tb0}_{tq}")
                    nc.scalar.activation(tn[:], x2[:, tb, :], AF.Identity,
                                         bias=st_nm[:, tb:tb + 1],
                                         scale=st_rs[:, tb:tb + 1])
                    for db in range(DB):
                        nc.tensor.transpose(trt[db // 4][:, db % 4, tq, :],
                                            tn[:, ts(db, P)], identB[:])
                for db in range(DB):
                    nc.vector.tensor_scalar(
                        H2T16[:, db, ds(tb0 * P, ntb * P)],
                        trt[db // 4][:, db % 4, 0:ntb, :].rearrange(
                            "p a b -> p (a b)"),
                        g2v[:, db:db + 1], c2v[:, db:db + 1],
                        OP.mult, OP.add)

            w1cache = {}

            def emit_ffn1_chunk(qr, fc, y1q):
                tb0, ntb = qr
                w = ntb * P
                if fc % 2 == 0:
                    w1c = bp.tile([P, 2, DB, P], BF, tag="w1c", bufs=3,
                                  name=f"w1c{tb0}_{fc}")
                    nc.sync.dma_start(w1c[:], w1_d[:, fc:fc + 2])
                    w1cache[0] = w1c
                w1c = w1cache[0]
                p = psS.tile([P, 512], F32, tag="mm", bufs=2,
                             name=f"f1{tb0}_{fc}")
                for db in range(DB):
                    nc.tensor.matmul(p[:, 0:w], w1c[:, fc % 2, db, :],
                                     H2T16[:, db, ds(tb0 * P, w)],
                                     start=(db == 0), stop=(db == DB - 1))
                if use_lrelu:
                    nc.scalar.activation(y1q[:, fc, 0:w], p[:, 0:w], AF.Prelu,
                                         bias=b1v[:, fc:fc + 1], alpha=0.01)
                else:
                    z = bp.tile([P, QW], F32, tag="zz", bufs=2,
                                name=f"z{tb0}_{fc}")
                    nc.scalar.activation(z[:, 0:w], p[:, 0:w], AF.Identity,
                                         bias=b1v[:, fc:fc + 1])
                    zs = bp.tile([P, QW], F32, tag="rb", bufs=2,
                                 name=f"zs{tb0}_{fc}")
                    nc.vector.tensor_scalar_mul(zs[:, 0:w], z[:, 0:w], 0.01)
                    nc.vector.tensor_tensor(y1q[:, fc, 0:w], z[:, 0:w],
                                            zs[:, 0:w], OP.max)

            def emit_ffn2_dt(qr, y1q, dt):
                tb0, ntb = qr
                fbw = 2
                if True:
                    pf = [psS.tile([P, 512], F32, tag="f2", bufs=2,
                                   name=f"f2{tb0}_{dt}_{tq}")
                          for tq in range(ntb)]
                    for fbp in range(FC // fbw):
                        w2c = bp.tile([P, fbw, 512], BF, tag="w2c", bufs=3,
                                      name=f"w2c{tb0}_{dt}_{fbp}")
                        nc.sync.dma_start(
                            w2c[:],
                            w2_d[:, fbw * fbp:fbw * fbp + fbw,
                                 ds(dt * 512, 512)])
                        for u in range(fbw):
                            fb = fbw * fbp + u
                            for tq in range(ntb):
                                nc.tensor.matmul(pf[tq][:],
                                                 y1q[:, fb, ts(tq, P)],
                                                 w2c[:, u, :],
                                                 start=(fb == 0), stop=False)
                    for tq in range(ntb):
                        tb = tb0 + tq
                        nc.tensor.matmul(pf[tq][:], onesRowR[:],
                                         b2rowR[:, ds(dt * 512, 512)],
                                         start=False, stop=True)
                        og = bp.tile([P, 512], F32, tag="og", bufs=2,
                                     name=f"og{tb0}_{dt}_{tq}")
                        nc.vector.tensor_tensor(og[:], pf[tq][:],
                                                x2[:, tb, ds(dt * 512, 512)],
                                                OP.add)
                        nc.scalar.dma_start(outr3[:, tb, ds(dt * 512, 512)],
                                            og[:])

            # pipeline: h-loop(q) interleaves FFN1(q-1); then FFN2(q-1),
            # proj(q), LN2(q).
            y1_prev = None
            QUARTERS = [(0, 2), (2, 2), (4, 2), (6, 2)]
            for qi, qr in enumerate(QUARTERS):
                prev = None
                for h in range(H):
                    ptsq = bp.tile([P, NT, QW], F8, tag="wqk", bufs=2,
                                   name=f"pts{qr[0]}_{h}")
                    emit_scores_exp(h, qr, ptsq)
                    if y1_prev is not None:
                        emit_ffn1_chunk(y1_prev[0], 2 * h, y1_prev[1])
                    elif qi == 0 and h < 2:
                        # all fh=1 V blocks and deferred Q(nh2=1) groups
                        # must precede the first attnV evac (attnN8 reuses
                        # HT8's region); they also fill the exp-bound q0
                        # bubble with PE work
                        for tb4 in range(4 * h, 4 * h + 4):
                            emit_v_block(tb4, 1, psS, ("mm", 2))
                    if prev is not None:
                        emit_attnv(prev[0], qr, prev[1])
                    if y1_prev is not None:
                        emit_ffn1_chunk(y1_prev[0], 2 * h + 1, y1_prev[1])
                    prev = (h, ptsq)
                emit_attnv(prev[0], qr, prev[1])
                if y1_prev is not None:
                    emit_ffn2_dt(y1_prev[0], y1_prev[1], 0)
                emit_proj(qr)
                if y1_prev is not None:
                    emit_ffn2_dt(y1_prev[0], y1_prev[1], 1)
                emit_ln2(qr)
                y1_prev = (qr, bp.tile([P, FC, QW], BF, tag="y1", bufs=2,
                                       name=f"y1_{qr[0]}"))
            # drain: FFN for the last quarter
            for fc in range(FC):
                emit_ffn1_chunk(y1_prev[0], fc, y1_prev[1])
            emit_ffn2_dt(y1_prev[0], y1_prev[1], 0)
            emit_ffn2_dt(y1_prev[0], y1_prev[1], 1)
            psS_cm.__exit__(None, None, None)
    nc.compile()
    return nc


def get_nc():
    global _CACHED_NC
    if _CACHED_NC is None:
        _CACHED_NC = build_nc()
    return _CACHED_NC


def _prep_weights(inputs):
    """Host-side layout + quantization (shared across cores)."""
    f32 = np.float32
    E4 = ml_dtypes.float8_e4m3
    BF16 = ml_dtypes.bfloat16
    q8 = lambda a: np.ascontiguousarray(a).astype(E4)
    qb = lambda a: np.ascontiguousarray(a).astype(BF16)
    g = lambda k: np.asarray(inputs[k], dtype=f32)

    Wq, Wk, Wv = g("Wq"), g("Wk"), g("Wv")
    Wp, W1, W2 = g("Wproj"), g("W1"), g("W2")
    bq, bk, bv = g("bq"), g("bk"), g("bv")

    def fold_qk(W):  # [H, D, HS] -> [dp, db, t, lh, i*32+hs32]
        a = W.reshape(4, 4, D // P, P, 2, 32)      # t i db dp lh h32
        return q8(a.transpose(3, 2, 0, 4, 1, 5).reshape(P, DB, 4, 2, P))

    def fold_bqk(b):  # [H, HS] -> [p=i*32+h32, t, lh]
        a = b.reshape(4, 4, 2, 32)                 # t i lh h32
        return np.ascontiguousarray(
            a.transpose(1, 3, 0, 2).reshape(P, 4, 2), dtype=f32)

    return {
        "wq8": fold_qk(Wq),
        "wk8": fold_qk(Wk),
        "wv8": q8(Wv.reshape(H, DB, P, HS).transpose(2, 1, 0, 3)
                  .reshape(P, DB, H * HS)),
        "wp8": q8(Wp.reshape(DB, P, D).transpose(1, 0, 2)),
        "w1h": qb(W1.reshape(DB, P, FC, P).transpose(1, 2, 0, 3)),
        "w2h": qb(W2.reshape(FC, P, D).transpose(1, 0, 2)),
        "bqf": fold_bqk(bq),
        "bkf": fold_bqk(bk),
        "bvpb": (bv.reshape(H * HS).astype(np.float64)
                 @ Wp.astype(np.float64)).astype(f32) + g("bproj"),
        "b1f": np.ascontiguousarray(g("b1").reshape(FC, P).T, dtype=f32),
        "b2": g("b2"),
        "ln1_g": g("ln1_g"), "ln1_b": g("ln1_b"),
        "ln2_g": g("ln2_g"), "ln2_b": g("ln2_b"),
    }


def kernel(**inputs):
    nc = get_nc()
    x = np.ascontiguousarray(np.asarray(inputs["x"], dtype=np.float32))
    B = x.shape[0]
    shared = _prep_weights(inputs)
    in_maps = [dict(shared, x=np.ascontiguousarray(x[b])) for b in range(B)]
    res = run_bass_kernel_spmd(nc, in_maps, list(range(B)))
    return np.stack([res.results[b]["out"] for b in range(B)], axis=0)


# revision 5
# speedup vs baseline: 1.0744x; 1.0093x over previous
"""Trainium2 Bass kernel v2 for the pre-LN transformer block (MHA + FFN).

Data-parallel over batch: 8 NeuronCores, one batch element each.
Attention side runs fp8 e4m3 with DoubleRow matmuls (2 k-tiles/instr at
0.5 cy/row): QKV projections, scores (folded 32-partition layout),
attn@V (+ softmax denominator via a ones-column DoubleRow matmul), and
the output projection. The FFN runs bf16 (fp8 there busts the 2e-2
accuracy gate). Softmax exp is computed once on ACT with a 1/4 output
scale folded in (cancels in normalization) so probs fit fp8 range.
Work is software-pipelined over 4 token quarters so the ACT-bound exp
phase overlaps the PE-bound FFN of the previous quarter.

Weights are pre-quantized and laid out host-side in kernel().
"""
import sys

for _p in ("/opt/trn_rl_repo", "/root/.axon_site/_ro/trn_rl_repo"):
    if _p not in sys.path:
        sys.path.insert(0, _p)

import numpy as np
import ml_dtypes
import concourse.bass as bass
import concourse.tile as tile
from concourse import bacc, mybir
from concourse.bass import ds, ts
from concourse.bass_utils import run_bass_kernel_spmd
from concourse.masks import make_identity

P = 128
N = 1024
D = 1024
H = 16
HS = 64
FF = 4096
NT = 8            # token tiles of 128
DB = 8            # d-blocks of 128
NQ = 4            # pipeline quarters over tokens
QW = N // NQ      # 256 tokens per quarter
FC = FF // P      # 32 f-chunks
LN_EPS = 1e-5
MLN4 = -1.3862943611198906  # ln(1/4): exp output scale, cancels in softmax

F32 = mybir.dt.float32
R = mybir.dt.float32r
BF = mybir.dt.bfloat16
F8 = mybir.dt.float8e4
AF = mybir.ActivationFunctionType
OP = mybir.AluOpType
DR = mybir.MatmulPerfMode.DoubleRow

_CACHED_NC = None


def build_nc(use_lrelu=True):
    nc = bacc.Bacc("TRN2", target_bir_lowering=False, debug=False, num_devices=8)

    x_d = nc.dram_tensor("x", [N, D], F32, kind="ExternalInput").ap()
    wq_d = nc.dram_tensor("wq8", [P, DB, 4, 2, P], F8, kind="ExternalInput").ap()
    wk_d = nc.dram_tensor("wk8", [P, DB, 4, 2, P], F8, kind="ExternalInput").ap()
    wv_d = nc.dram_tensor("wv8", [P, DB, H * HS], F8, kind="ExternalInput").ap()
    wp_d = nc.dram_tensor("wp8", [P, DB, D], F8, kind="ExternalInput").ap()
    w1_d = nc.dram_tensor("w1h", [P, FC, DB, P], BF, kind="ExternalInput").ap()
    w2_d = nc.dram_tensor("w2h", [P, FC, D], BF, kind="ExternalInput").ap()
    bq_d = nc.dram_tensor("bqf", [P, 4, 2], F32, kind="ExternalInput").ap()
    bk_d = nc.dram_tensor("bkf", [P, 4, 2], F32, kind="ExternalInput").ap()
    bvpb_d = nc.dram_tensor("bvpb", [D], F32, kind="ExternalInput").ap()
    b1_d = nc.dram_tensor("b1f", [P, FC], F32, kind="ExternalInput").ap()
    b2_d = nc.dram_tensor("b2", [D], F32, kind="ExternalInput").ap()
    g1_d = nc.dram_tensor("ln1_g", [D], F32, kind="ExternalInput").ap()
    c1_d = nc.dram_tensor("ln1_b", [D], F32, kind="ExternalInput").ap()
    g2_d = nc.dram_tensor("ln2_g", [D], F32, kind="ExternalInput").ap()
    c2_d = nc.dram_tensor("ln2_b", [D], F32, kind="ExternalInput").ap()
    out_d = nc.dram_tensor("out", [N, D], F32, kind="ExternalOutput").ap()

    xr3 = x_d.rearrange("(t p) d -> p t d", p=P)
    outr3 = out_d.rearrange("(t p) d -> p t d", p=P)

    with tile.TileContext(nc) as tc:
        with tc.tile_pool(name="cn", bufs=1) as cp, \
             tc.tile_pool(name="big", bufs=1) as bp:
            # ---------------- constants / small vectors ----------------
            identB = cp.tile([P, P], BF)
            make_identity(nc, identB[:])
            onesPc = cp.tile([P, 1], F32)
            nc.vector.memset(onesPc[:], 1.0)
            onesF = bp.tile([1, QW], F32, tag="rc", bufs=2, name="onesF")
            nc.vector.memset(onesF[:], 1.0)
            onesRow = cp.tile([1, P], R)
            nc.vector.tensor_copy(onesRow[:], onesF[:, 0:P])
            onesRowR = onesRow
            mln4 = cp.tile([P, 1], F32)
            nc.vector.memset(mln4[:], MLN4)

            xsb = bp.tile([P, NT, D], F32, tag="xsb", name="xsb")
            wq8s = bp.tile([P, DB, 4, 2, P], F8, tag="wqk", bufs=2, name="wq8s")
            wk8s = bp.tile([P, DB, 4, 2, P], F8, tag="wqk", bufs=2, name="wk8s")
            wv8s = bp.tile([P, DB, H * HS], F8, tag="y1", bufs=2, name="wv8s")
            wp8s = bp.tile([P, DB, D], F8, tag="wp", name="wp8s")
            for tb in range(NT):
                nc.sync.dma_start(xsb[:, tb, :], xr3[:, tb, :])
            g1v = cp.tile([P, DB], F32)
            nc.sync.dma_start(g1v[:], g1_d.rearrange("(b p) -> p b", p=P))
            c1v = cp.tile([P, DB], F32)
            nc.sync.dma_start(c1v[:], c1_d.rearrange("(b p) -> p b", p=P))
            g2v = cp.tile([P, DB], F32)
            nc.sync.dma_start(g2v[:], g2_d.rearrange("(b p) -> p b", p=P))
            c2v = cp.tile([P, DB], F32)
            nc.sync.dma_start(c2v[:], c2_d.rearrange("(b p) -> p b", p=P))
            bqv = cp.tile([P, 4, 2], F32)
            nc.sync.dma_start(bqv[:], bq_d)
            bkv = cp.tile([P, 4, 2], F32)
            nc.sync.dma_start(bkv[:], bk_d)
            b1v = cp.tile([P, FC], F32)
            nc.sync.dma_start(b1v[:], b1_d)
            b2row = cp.tile([1, D], R)
            nc.sync.dma_start(b2row[:], b2_d[None, :].bitcast(R))
            bvp = cp.tile([1, D], R)       # bv @ Wproj + bproj (host)
            nc.sync.dma_start(bvp[:], bvpb_d[None, :].bitcast(R))
            nc.sync.dma_start(wq8s[:], wq_d)
            nc.sync.dma_start(wk8s[:], wk_d)
            nc.sync.dma_start(wv8s[:], wv_d)
            nc.sync.dma_start(wp8s[:], wp_d)

            # LN stats scratch
            st_sum = cp.tile([P, NT], F32)
            st_sq = cp.tile([P, NT], F32)
            st_var = cp.tile([P, NT], F32)
            st_rs = cp.tile([P, NT], F32)
            st_nm = cp.tile([P, NT], F32)
            st_vh = cp.tile([P, NT], F32)
            st_t = cp.tile([P, NT], F32)
            st_ih = cp.tile([P, NT], mybir.dt.int32)

            def ln_reduce(src, tb, pfx):
                t1 = (tb, tb + 1)
                nc.vector.reduce_sum(st_sum[:, t1[0]:t1[1]], src,
                                     axis=mybir.AxisListType.X)
                dump = bp.tile([P, D], BF, tag="tn", bufs=2,
                               name=f"dmp{pfx}{tb}")
                nc.scalar.activation(dump[:], src, AF.Square,
                                     accum_out=st_sq[:, t1[0]:t1[1]])

            def ln_chain(lo, hi):
                """Vectorized var/rsqrt chain over st[:, lo:hi] (DVE-only
                rsqrt: bit hack + 2 Newton steps). Fills st_rs, st_nm."""
                sm = st_sum[:, lo:hi]
                sq = st_sq[:, lo:hi]
                var = st_var[:, lo:hi]
                rs = st_rs[:, lo:hi]
                nm = st_nm[:, lo:hi]
                ih = st_ih[:, lo:hi]
                vh = st_vh[:, lo:hi]
                tt = st_t[:, lo:hi]
                i32 = mybir.dt.int32
                # var = sq/D - (sum/D)^2 + eps
                nc.vector.tensor_tensor(tt, sm, sm, OP.mult)
                nc.vector.tensor_scalar(var, tt, -1.0 / (D * D), LN_EPS,
                                        OP.mult, OP.add)
                nc.vector.tensor_scalar_mul(tt, sq, 1.0 / D)
                nc.vector.tensor_tensor(var, var, tt, OP.add)
                nc.vector.tensor_scalar(ih, var.bitcast(i32), 1, None,
                                        OP.arith_shift_right)
                nc.vector.tensor_scalar(rs.bitcast(i32), ih, -1,
                                        0x5F3759DF, OP.mult, OP.add)
                nc.vector.tensor_scalar_mul(vh, var, -0.5)
                for _ in range(2):
                    nc.vector.tensor_tensor(tt, rs, rs, OP.mult)
                    nc.vector.tensor_tensor(tt, tt, vh, OP.mult)
                    nc.vector.tensor_scalar_add(tt, tt, 1.5)
                    nc.vector.tensor_tensor(rs, rs, tt, OP.mult)
                nc.vector.tensor_tensor(nm, sm, rs, OP.mult)
                nc.vector.tensor_scalar_mul(nm, nm, -1.0 / D)

            # ================= Phase A+B: LN1 -> HT8, QKV (fp8 DR) =========
            HT8 = bp.tile([P, DB, N], F8, tag="ht", name="HT8")
            Qf8 = bp.tile([P, 4, 2, N], F8, tag="qf", name="Qf8")
            Kf8 = bp.tile([P, 4, 2, N], F8, tag="kf", name="Kf8")
            V8 = bp.tile([P, NT, H, HS + 1], F8, tag="v8", name="V8")
            nc.vector.tensor_copy(
                V8[:, :, :, HS:HS + 1],
                onesPc[:, None, :].to_broadcast([P, NT, H, 1]))
            psB_cm = tc.tile_pool(name="psB", bufs=1, space="PSUM")
            psB = psB_cm.__enter__()

            def emit_ln1_half(half):
                trt = []
                for g in range(4):
                    pt = psB.tile([P, 512], F32, tag="mm", bufs=4,
                                  name=f"l1t{half}_{g}")
                    trt.append(pt.bitcast(BF).rearrange(
                        "p (a b c) -> p a b c", a=2, b=4))
                if half == 0:
                    ln_reduce(xsb[:, 0, :], 0, "a")
                    ln_chain(0, 1)
                    for tq in range(1, 4):
                        ln_reduce(xsb[:, tq, :], tq, "a")
                    ln_chain(1, 4)
                else:
                    for tq in range(4):
                        ln_reduce(xsb[:, 4 + tq, :], 4 + tq, "a")
                    ln_chain(4, 8)
                for tq in range(4):
                    tb = half * 4 + tq
                    tn = bp.tile([P, D], BF, tag="tn", bufs=2, name=f"tn{tb}")
                    nc.scalar.activation(tn[:], xsb[:, tb, :], AF.Identity,
                                         bias=st_nm[:, tb:tb + 1],
                                         scale=st_rs[:, tb:tb + 1])
                    for db in range(DB):
                        nc.tensor.transpose(trt[db // 2][:, db % 2, tq, :],
                                            tn[:, ts(db, P)], identB[:])
                for db in range(DB):
                    nc.vector.tensor_scalar(
                        HT8[:, db, ds(half * 512, 512)],
                        trt[db // 2][:, db % 2, :, :].rearrange(
                            "p a b -> p (a b)"),
                        g1v[:, db:db + 1], c1v[:, db:db + 1],
                        OP.mult, OP.add)

            def emit_v_block(tb, fh, pool, tagbufs):
                p = pool.tile([P, 512], F32, tag=tagbufs[0], bufs=tagbufs[1],
                              name=f"pv{tb}_{fh}")
                for j in range(4):
                    nc.tensor.matmul(
                        p[:], HT8[:, 2 * j:2 * j + 2, ts(tb, P)],
                        wv8s[:, 2 * j:2 * j + 2, ds(fh * 512, 512)],
                        start=(j == 0), stop=(j == 3), perf_mode=DR)
                if fh == 0:   # phase B: DVE is loaded, use ACT
                    nc.scalar.copy(
                        V8[:, tb, 0:8, 0:HS],
                        p.rearrange("p (h s) -> p h s", s=HS))
                else:
                    nc.vector.tensor_copy(
                        V8[:, tb, 8:16, 0:HS],
                        p.rearrange("p (h s) -> p h s", s=HS))

            def emit_qk_group(wsrc, bsrc, dstf, pfx, t, lh, nh2,
                              pool=None, bufs=4, on_act=True):
                pool = pool or psB
                p = pool.tile([P, 512], F32, tag="mm", bufs=bufs,
                              name=f"p{pfx}{t}{lh}{nh2}")
                for j in range(4):
                    nc.tensor.matmul(
                        p[:], wsrc[:, 2 * j:2 * j + 2, t, lh, :],
                        HT8[:, 2 * j:2 * j + 2, ds(nh2 * 512, 512)],
                        start=(j == 0), stop=(j == 3), perf_mode=DR)
                if on_act:
                    nc.scalar.activation(
                        dstf[:, t, lh, ds(nh2 * 512, 512)], p[:],
                        AF.Identity, bias=bsrc[:, t, lh:lh + 1])
                else:
                    nc.vector.tensor_scalar_add(
                        dstf[:, t, lh, ds(nh2 * 512, 512)], p[:],
                        bsrc[:, t, lh:lh + 1])

            # LN1 half0 -> half0-token matmuls -> LN1 half1 -> rest
            for nh2 in range(2):
                emit_ln1_half(nh2)
                for tb in range(4 * nh2, 4 * nh2 + 4):
                    emit_v_block(tb, 0, psB, ("mm", 4))
                for (wsrc, bsrc, dstf, pfx) in ((wq8s, bqv, Qf8, "q"),
                                                (wk8s, bkv, Kf8, "k")):
                    for t in range(4):
                        for lh in range(2):
                            emit_qk_group(wsrc, bsrc, dstf, pfx, t, lh, nh2,
                                          on_act=(pfx == "q"))

            bvpR = bvp
            b2rowR = b2row
            psB_cm.__exit__(None, None, None)

            # ============== Steady state: per-quarter pipeline =============
            attnN8 = bp.tile([P, DB, N], F8, tag="ht", name="attnN8")
            x2 = bp.tile([P, NT, D], F32, tag="x2", name="x2")
            H2T16 = bp.tile([P, DB, N], BF, tag="h2t", name="H2T16")

            psS_cm = tc.tile_pool(name="psS", bufs=1, space="PSUM")
            psS = psS_cm.__enter__()

            def emit_scores_exp(h, qr, ptsq):
                tb0, ntb = qr
                w = ntb * P
                t, i = h // 4, h % 4
                ip = ds(32 * i, 32)
                for jp in range(2):
                    psc = psS.tile([P, 4, QW], F32, tag="sc", bufs=2,
                                   name=f"sc{h}_{tb0}_{jp}")
                    for u in range(4):
                        mt = 4 * jp + u
                        nc.tensor.matmul(
                            psc[:, u, 0:w],
                            Kf8[ip, t, :, ts(mt, P)],
                            Qf8[ip, t, :, ds(tb0 * P, w)],
                            start=True, stop=True, perf_mode=DR,
                            tile_position=(32 * i, 0))
                    nc.scalar.activation(ptsq[:, 4 * jp:4 * jp + 4, 0:w],
                                         psc[:, :, 0:w], AF.Exp,
                                         bias=mln4[:], scale=0.125)

            def emit_attnv(h, qr, ptsq):
                tb0, ntb = qr
                w = ntb * P
                pavt = psS.tile([P, 512], F32, tag="mm", bufs=2,
                                name=f"av{h}_{tb0}")
                pav = pavt[:, 0:w]
                for j in range(4):
                    sl = ptsq[:, 2 * j:2 * j + 2, 0:w]
                    nc.tensor.matmul(pav[0:65, :],
                                     V8[:, 2 * j:2 * j + 2, h, :],
                                     sl, start=(j == 0), stop=(j == 3),
                                     perf_mode=DR)
                rec = bp.tile([1, QW], F32, tag="rc", bufs=2,
                              name=f"rc{h}_{tb0}")
                nc.vector.reciprocal(rec[:, 0:w], pav[64:65, :])
                rbs = bp.tile([64, QW], F32, tag="rb", bufs=2,
                              name=f"rb{h}_{tb0}")
                nc.gpsimd.partition_broadcast(rbs[:, 0:w], rec[:, 0:w])
                nc.vector.tensor_tensor(
                    attnN8[ds(64 * (h % 2), 64), h // 2, ds(tb0 * P, w)],
                    pav[0:64, :], rbs[:, 0:w], OP.mult)

            def emit_proj(qr):
                tb0, ntb = qr
                for tq in range(ntb):
                    tb = tb0 + tq
                    for dt in range(2):
                        p = psS.tile([P, 512], F32, tag="mm", bufs=2,
                                     name=f"pp{tb}_{dt}")
                        for j in range(4):
                            nc.tensor.matmul(
                                p[:], attnN8[:, 2 * j:2 * j + 2, ts(tb, P)],
                                wp8s[:, 2 * j:2 * j + 2, ds(dt * 512, 512)],
                                start=(j == 0), stop=False, perf_mode=DR)
                        # rank-1 (bv@Wp + bproj) row folded into the group
                        nc.tensor.matmul(p[:], onesRowR[:],
                                         bvpR[:, ds(dt * 512, 512)],
                                         start=False, stop=True)
                        nc.vector.tensor_tensor(
                            x2[:, tb, ds(dt * 512, 512)], p[:],
                            xsb[:, tb, ds(dt * 512, 512)], OP.add)

            def emit_ln2(qr):
                tb0, ntb = qr
                trt = []
                for g in range(2):
                    pt = psS.tile([P, 512], F32, tag="mm", bufs=2,
                                  name=f"l2t{tb0}_{g}")
                    trt.append(pt.bitcast(BF).rearrange(
                        "p (a b c) -> p a b c", a=4, b=2))
                for tq in range(ntb):
                    ln_reduce(x2[:, tb0 + tq, :], tb0 + tq, "e")
                ln_chain(tb0, tb0 + ntb)
                for tq in range(ntb):
                    tb = tb0 + tq
                    tn = bp.tile([P, D], BF, tag="tn", bufs=2,
                                 name=f"t2{tb0}_{tq}")
                    nc.scalar.activation(tn[:], x2[:, tb, :], AF.Identity,
                                         bias=st_nm[:, tb:tb + 1],
                                         scale=st_rs[:, tb:tb + 1])
                    for db in range(DB):
                        nc.tensor.transpose(trt[db // 4][:, db % 4, tq, :],
                                            tn[:, ts(db, P)], identB[:])
                for db in range(DB):
                    nc.vector.tensor_scalar(
                        H2T16[:, db, ds(tb0 * P, ntb * P)],
                        trt[db // 4][:, db % 4, 0:ntb, :].rearrange(
                            "p a b -> p (a b)"),
                        g2v[:, db:db + 1], c2v[:, db:db + 1],
                        OP.mult, OP.add)

            w1cache = {}

            def emit_ffn1_chunk(qr, fc, y1q):
                tb0, ntb = qr
                w = ntb * P
                if fc % 2 == 0:
                    w1c = bp.tile([P, 2, DB, P], BF, tag="w1c", bufs=2,
                                  name=f"w1c{tb0}_{fc}")
                    nc.sync.dma_start(w1c[:], w1_d[:, fc:fc + 2])
                    w1cache[0] = w1c
                w1c = w1cache[0]
                p = psS.tile([P, 512], F32, tag="mm", bufs=2,
                             name=f"f1{tb0}_{fc}")
                for db in range(DB):
                    nc.tensor.matmul(p[:, 0:w], w1c[:, fc % 2, db, :],
                                     H2T16[:, db, ds(tb0 * P, w)],
                                     start=(db == 0), stop=(db == DB - 1))
                if use_lrelu:
                    nc.scalar.activation(y1q[:, fc, 0:w], p[:, 0:w], AF.Prelu,
                                         bias=b1v[:, fc:fc + 1], alpha=0.01)
                else:
                    z = bp.tile([P, QW], F32, tag="zz", bufs=2,
                                name=f"z{tb0}_{fc}")
                    nc.scalar.activation(z[:, 0:w], p[:, 0:w], AF.Identity,
                                         bias=b1v[:, fc:fc + 1])
                    zs = bp.tile([P, QW], F32, tag="rb", bufs=2,
                                 name=f"zs{tb0}_{fc}")
                    nc.vector.tensor_scalar_mul(zs[:, 0:w], z[:, 0:w], 0.01)
                    nc.vector.tensor_tensor(y1q[:, fc, 0:w], z[:, 0:w],
                                            zs[:, 0:w], OP.max)

            def emit_ffn2_dt(qr, y1q, dt):
                tb0, ntb = qr
                fbw = 2
                if True:
                    pf = [psS.tile([P, 512], F32, tag="f2", bufs=2,
                                   name=f"f2{tb0}_{dt}_{tq}")
                          for tq in range(ntb)]
                    for fbp in range(FC // fbw):
                        w2c = bp.tile([P, fbw, 512], BF, tag="w2c", bufs=5,
                                      name=f"w2c{tb0}_{dt}_{fbp}")
                        nc.sync.dma_start(
                            w2c[:],
                            w2_d[:, fbw * fbp:fbw * fbp + fbw,
                                 ds(dt * 512, 512)])
                        for u in range(fbw):
                            fb = fbw * fbp + u
                            for tq in range(ntb):
                                nc.tensor.matmul(pf[tq][:],
                                                 y1q[:, fb, ts(tq, P)],
                                                 w2c[:, u, :],
                                                 start=(fb == 0), stop=False)
                    for tq in range(ntb):
                        tb = tb0 + tq
                        nc.tensor.matmul(pf[tq][:], onesRowR[:],
                                         b2rowR[:, ds(dt * 512, 512)],
                                         start=False, stop=True)
                        og = bp.tile([P, 512], F32, tag="og", bufs=2,
                                     name=f"og{tb0}_{dt}_{tq}")
                        nc.vector.tensor_tensor(og[:], pf[tq][:],
                                                x2[:, tb, ds(dt * 512, 512)],
                                                OP.add)
                        nc.scalar.dma_start(outr3[:, tb, ds(dt * 512, 512)],
                                            og[:])

            # pipeline: h-loop(q) interleaves FFN1(q-1); then FFN2(q-1),
            # proj(q), LN2(q).
            y1_prev = None
            QUARTERS = [(0, 2), (2, 2), (4, 2), (6, 2)]
            for qi, qr in enumerate(QUARTERS):
                prev = None
                for h in range(H):
                    ptsq = bp.tile([P, NT, QW], F8, tag="wqk", bufs=2,
                                   name=f"pts{qr[0]}_{h}")
                    emit_scores_exp(h, qr, ptsq)
                    if y1_prev is not None:
                        emit_ffn1_chunk(y1_prev[0], 2 * h, y1_prev[1])
                    elif qi == 0 and h < 2:
                        # all fh=1 V blocks and deferred Q(nh2=1) groups
                        # must precede the first attnV evac (attnN8 reuses
                        # HT8's region); they also fill the exp-bound q0
                        # bubble with PE work
                        for tb4 in range(4 * h, 4 * h + 4):
                            emit_v_block(tb4, 1, psS, ("mm", 2))
                    if prev is not None:
                        emit_attnv(prev[0], qr, prev[1])
                    if y1_prev is not None:
                        emit_ffn1_chunk(y1_prev[0], 2 * h + 1, y1_prev[1])
                    prev = (h, ptsq)
                emit_attnv(prev[0], qr, prev[1])
                if y1_prev is not None:
                    emit_ffn2_dt(y1_prev[0], y1_prev[1], 0)
                emit_proj(qr)
                if y1_prev is not None:
                    emit_ffn2_dt(y1_prev[0], y1_prev[1], 1)
                emit_ln2(qr)
                y1_prev = (qr, bp.tile([P, FC, QW], BF, tag="y1", bufs=2,
                                       name=f"y1_{qr[0]}"))
            # drain: FFN for the last quarter
            for fc in range(FC):
                emit_ffn1_chunk(y1_prev[0], fc, y1_prev[1])
            emit_ffn2_dt(y1_prev[0], y1_prev[1], 0)
            emit_ffn2_dt(y1_prev[0], y1_prev[1], 1)
            psS_cm.__exit__(None, None, None)
    nc.compile()
    return nc


def get_nc():
    global _CACHED_NC
    if _CACHED_NC is None:
        _CACHED_NC = build_nc()
    return _CACHED_NC


def _prep_weights(inputs):
    """Host-side layout + quantization (shared across cores)."""
    f32 = np.float32
    E4 = ml_dtypes.float8_e4m3
    BF16 = ml_dtypes.bfloat16
    q8 = lambda a: np.ascontiguousarray(a).astype(E4)
    qb = lambda a: np.ascontiguousarray(a).astype(BF16)
    g = lambda k: np.asarray(inputs[k], dtype=f32)

    Wq, Wk, Wv = g("Wq"), g("Wk"), g("Wv")
    Wp, W1, W2 = g("Wproj"), g("W1"), g("W2")
    bq, bk, bv = g("bq"), g("bk"), g("bv")

    def fold_qk(W):  # [H, D, HS] -> [dp, db, t, lh, i*32+hs32]
        a = W.reshape(4, 4, D // P, P, 2, 32)      # t i db dp lh h32
        return q8(a.transpose(3, 2, 0, 4, 1, 5).reshape(P, DB, 4, 2, P))

    def fold_bqk(b):  # [H, HS] -> [p=i*32+h32, t, lh]
        a = b.reshape(4, 4, 2, 32)                 # t i lh h32
        return np.ascontiguousarray(
            a.transpose(1, 3, 0, 2).reshape(P, 4, 2), dtype=f32)

    return {
        "wq8": fold_qk(Wq),
        "wk8": fold_qk(Wk),
        "wv8": q8(Wv.reshape(H, DB, P, HS).transpose(2, 1, 0, 3)
                  .reshape(P, DB, H * HS)),
        "wp8": q8(Wp.reshape(DB, P, D).transpose(1, 0, 2)),
        "w1h": qb(W1.reshape(DB, P, FC, P).transpose(1, 2, 0, 3)),
        "w2h": qb(W2.reshape(FC, P, D).transpose(1, 0, 2)),
        "bqf": fold_bqk(bq),
        "bkf": fold_bqk(bk),
        "bvpb": (bv.reshape(H * HS).astype(np.float64)
                 @ Wp.astype(np.float64)).astype(f32) + g("bproj"),
        "b1f": np.ascontiguousarray(g("b1").reshape(FC, P).T, dtype=f32),
        "b2": g("b2"),
        "ln1_g": g("ln1_g"), "ln1_b": g("ln1_b"),
        "ln2_g": g("ln2_g"), "ln2_b": g("ln2_b"),
    }


def kernel(**inputs):
    nc = get_nc()
    x = np.ascontiguousarray(np.asarray(inputs["x"], dtype=np.float32))
    B = x.shape[0]
    shared = _prep_weights(inputs)
    in_maps = [dict(shared, x=np.ascontiguousarray(x[b])) for b in range(B)]
    res = run_bass_kernel_spmd(nc, in_maps, list(range(B)))
    return np.stack([res.results[b]["out"] for b in range(B)], axis=0)
